# revision 1
# baseline (speedup 1.0000x reference)
"""CTC loss (nn_CTCLoss) Trainium2 Bass kernel.

Sharding: data-parallel over batch N across 8 cores (8 samples/core).

Per core:
  * Bulk pass: stream the (T, NL=8, C) f32 shard through SBUF as
    [128=(n,t16), C] tiles.  ScalarE computes exp() with a fused per-row
    accumulate (softmax denominator per (t, n)); GpSimd ap_gather pulls
    the S=2L+1 extended-label columns per sample (index lists are
    host-prepared; each 16-partition group shares one sample's list);
    ScalarE exponentiates the gathered logits (bias=+SHIFT).  q rows plus
    the accumulator column bounce through a DRAM scratch to move t from
    partitions onto the free axis in DP order (the backward direction is
    stored time-reversed and gathered state-reversed, so both chains read
    forward).
  * DP pass on VectorE in the probability domain: forward chain on
    partitions 0-7, time/state-reversed backward chain on partitions
    8-15, advanced together by 3 (no label repeats) or 4 (general)
    instructions per time step; T/2 sequential steps per chain.  The
    softmax denominator is folded in per step via the scalar slot of
    scalar_tensor_tensor (scalar = 1/acc, periodically also carrying a
    renormalisation factor).
  * Final alpha/beta states plus renorm maxima (16 x ~70 floats/core) go
    back to the host, which stitches the chains at the midpoint in
    float64 and takes the batch mean.
"""

import sys

import numpy as np

for _p in ("/root/.axon_site", "/root/.axon_site/_ro/trn_rl_repo", "/opt/trn_rl_repo"):
    if _p not in sys.path:
        sys.path.append(_p)

NCORES = 8
NL = 8                   # samples per core
TG = 16                  # time steps per tile group (128 = NL * TG partitions)
BLANK = 0
PAD = 2                  # leading zero pad columns in DP tiles

# problem dims (the graded configuration)
T, N, C, L = 256, 64, 4096, 32


def _derived(T_, C_, L_, use_renorm=True):
    S_ = 2 * L_ + 1
    GW_ = S_ + 1                       # q row + acc column in DRAM scratch
    NIDX_ = ((S_ + 15) // 16) * 16     # gather list length (%16 == 0)
    NPAIR_ = T_ // (2 * TG)
    TH_ = T_ // 2                      # steps per chain
    if use_renorm:
        SHIFT_ = float(np.log(C_) + 0.5)
        renorm_ = tuple(range(31, TH_ - 1, 32))
    else:
        # -1.0 cancels the ~e^1 per-step growth of the summed mass; the
        # chain then stays inside fp32 range with no renorms at all
        SHIFT_ = float(np.log(C_) + 0.5) - 1.0
        renorm_ = ()
    return S_, GW_, NIDX_, NPAIR_, TH_, SHIFT_, renorm_


# ----------------------------------------------------------------------------
# host-side helpers
# ----------------------------------------------------------------------------

def _ext_labels(t2d, S_):
    ext = np.zeros((t2d.shape[0], S_), np.int64)
    ext[:, 1::2] = t2d
    return ext


def _skip_mask(ext):
    sidx = np.arange(ext.shape[1])
    return (
        (sidx[None, :] >= 2)
        & (ext != BLANK)
        & (ext != np.roll(ext, 2, axis=1))
    )


def _ref_numpy(preds, t2d, pred_lengths, target_lengths):
    """float64 port of the reference (fallback path)."""
    preds = preds.astype(np.float64)
    Tn, n = preds.shape[0], preds.shape[1]
    S_ = 2 * t2d.shape[1] + 1
    mx = preds.max(axis=2, keepdims=True)
    lp = preds - mx - np.log(np.exp(preds - mx).sum(axis=2, keepdims=True))
    ext = _ext_labels(t2d, S_)
    lpe = lp[:, np.arange(n)[:, None], ext]
    skip_ok = _skip_mask(ext)
    NEGI = -1e30
    sidx = np.arange(S_)
    valid = sidx[None, :] < (2 * target_lengths[:, None] + 1)
    alpha = np.full((n, S_), NEGI)
    alpha[:, 0] = lpe[0, :, 0]
    alpha[:, 1] = np.where(target_lengths > 0, lpe[0, :, 1], NEGI)
    alpha = np.where(valid, alpha, NEGI)

    def lse(*a):
        m = np.maximum.reduce(a)
        m = np.where(np.isfinite(m), m, 0.0)
        return m + np.log(sum(np.exp(x - m) for x in a))

    for t in range(1, Tn):
        a2 = np.concatenate([np.full((n, 1), NEGI), alpha[:, :-1]], 1)
        a3 = np.concatenate([np.full((n, 2), NEGI), alpha[:, :-2]], 1)
        a3 = np.where(skip_ok, a3, NEGI)
        new = np.where(valid, lse(alpha, a2, a3) + lpe[t], NEGI)
        active = (t < pred_lengths)[:, None]
        alpha = np.where(active, new, alpha)
    end = 2 * target_lengths
    a_last = alpha[np.arange(n), end]
    a_prev = alpha[np.arange(n), np.maximum(end - 1, 0)]
    a_prev = np.where(target_lengths > 0, a_prev, NEGI)
    nll = -lse(a_last, a_prev)
    nll = np.where(np.isfinite(nll) & (nll < 1e29), nll, 0.0)
    return np.float32(np.mean(nll / np.maximum(target_lengths, 1)))


def _wrap_idx(lists, NIDX_):
    """lists: (NL, NIDX) int -> ap_gather wrapped layout [128, NIDX//16]."""
    out = np.zeros((128, NIDX_ // 16), np.int16)
    for g in range(NL):
        for j in range(NIDX_):
            out[g * 16 + (j % 16), j // 16] = lists[g, j]
    return out


# ----------------------------------------------------------------------------
# kernel builder
# ----------------------------------------------------------------------------

_NC_CACHE = {}


def _build(use_masks, use_renorm, dims):
    T_, C_, L_ = dims
    S_, GW_, NIDX_, NPAIR_, TH_, SHIFT_, renorm_steps = _derived(
        T_, C_, L_, use_renorm)

    import concourse.bacc as bacc
    import concourse.bass as bass
    import concourse.tile as tile
    from concourse import library_config, mybir

    f32 = mybir.dt.float32
    Alu = mybir.AluOpType
    Act = mybir.ActivationFunctionType

    nc = bacc.Bacc("TRN2", target_bir_lowering=False, debug=False)
    # shard pre-tiled on host: [tile-group, n, t16, c] so each [128, C] tile
    # load reads 128 consecutive 16KB rows (full HBM bandwidth)
    px = nc.dram_tensor("px", [T_ // TG, NL, TG, C_], f32,
                        kind="ExternalInput")
    idx_f = nc.dram_tensor("idx_f", [128, NIDX_ // 16], mybir.dt.int16,
                           kind="ExternalInput")
    idx_b = nc.dram_tensor("idx_b", [128, NIDX_ // 16], mybir.dt.int16,
                           kind="ExternalInput")
    if use_masks:
        maskd = nc.dram_tensor("maskd", [16, PAD + S_], f32,
                               kind="ExternalInput")
    res = nc.dram_tensor("res", [2, 16, PAD + S_], f32, kind="ExternalOutput")

    with tile.TileContext(nc) as tc:
        with (
            tc.tile_pool(name="main", bufs=3) as main_pool,
            tc.tile_pool(name="scr", bufs=1) as scr_pool,
            tc.tile_pool(name="qslab", bufs=4) as qslab_pool,
            tc.tile_pool(name="gout", bufs=4) as gout_pool,
            tc.tile_pool(name="qc", bufs=NPAIR_) as qc_pool,
            tc.tile_pool(name="sc", bufs=NPAIR_) as sc_pool,
            tc.tile_pool(name="single", bufs=1) as single,
        ):
            nc.gpsimd.load_library(library_config.ap_gather)

            ixf = single.tile([128, NIDX_ // 16], mybir.dt.int16, tag="ixf")
            ixb = single.tile([128, NIDX_ // 16], mybir.dt.int16, tag="ixb")
            nc.scalar.dma_start(out=ixf[:], in_=idx_f[:])
            nc.scalar.dma_start(out=ixb[:], in_=idx_b[:])
            if use_masks:
                msk = single.tile([16, PAD + S_], f32, tag="msk")
                nc.scalar.dma_start(out=msk[:], in_=maskd[:])

            shiftb = single.tile([128, 1], f32, tag="shiftb")
            nc.vector.memset(shiftb[:], SHIFT_)

            A = single.tile([16, PAD + S_], f32, tag="A")
            t1 = single.tile([16, PAD + S_], f32, tag="t1")
            nc.vector.memset(A[:], 0.0)
            nc.vector.memset(t1[:], 0.0)
            if use_masks:
                am = single.tile([16, PAD + S_], f32, tag="am")
                nc.vector.memset(am[:], 0.0)
            Rbuf = single.tile([16, 4], f32, tag="R")
            nc.vector.memset(Rbuf[:], 1.0)
            rinv = single.tile([16, 1], f32, tag="rinv")
            patch = single.tile([16, 1], f32, tag="patch")

            pending_renorm = False
            for j in range(NPAIR_):
                tg_f, tg_b = j, 2 * NPAIR_ - 1 - j
                if j == 0:
                    # first pair split in two so the DP-feeding chain starts
                    # as soon as the forward half lands
                    mtp = main_pool.tile([128, 2 * C_], f32, tag="mt")
                    nc.sync.dma_start(
                        out=mtp[:, 0:C_],
                        in_=px[tg_f].rearrange("n t c -> (n t) c"))
                    nc.sync.dma_start(
                        out=mtp[:, C_:2 * C_],
                        in_=px[tg_b].rearrange("n t c -> (n t) c"))
                    halves = [(0, mtp[:, 0:C_]), (1, mtp[:, C_:2 * C_])]
                else:
                    # one 4 MB DMA per pair: halves are the fwd and bwd
                    # groups, each 128 consecutive 16KB rows
                    mtp = main_pool.tile([128, 2 * C_], f32, tag="mt")
                    src = bass.AP(
                        tensor=px[:].tensor,
                        offset=px[:].offset + tg_f * 128 * C_,
                        ap=[[C_, 128], [(tg_b - tg_f) * 128 * C_, 2], [1, C_]],
                    )
                    dst = bass.AP(
                        tensor=mtp[:].tensor,
                        offset=mtp[:].offset,
                        ap=[[2 * C_, 128], [C_, 2], [1, C_]],
                    )
                    nc.sync.dma_start(out=dst, in_=src)
                    halves = [(0, mtp[:, 0:C_]), (1, mtp[:, C_:2 * C_])]

                # chunk tile first: both q slabs transpose straight into it
                qc = qc_pool.tile([16, TG, GW_], f32, tag="qc")

                for dr, mt in halves:
                    qs = qslab_pool.tile([128, GW_], f32, tag="qs")
                    scr = scr_pool.tile([128, C_], f32, tag="scr")
                    # exp + fused row-sum (softmax denominator -> col S)
                    nc.scalar.activation(scr[:], mt, Act.Exp,
                                         bias=0.0, scale=1.0,
                                         accum_out=qs[:, S_:S_ + 1])
                    go = gout_pool.tile([128, NIDX_], f32, tag="go")
                    ix = ixf if dr == 0 else ixb
                    nc.gpsimd.ap_gather(go[:], mt, ix[:],
                                        channels=128, num_elems=C_, d=1,
                                        num_idxs=NIDX_)
                    nc.scalar.activation(qs[:, 0:S_], go[:, 0:S_], Act.Exp,
                                         bias=shiftb[:, 0:1], scale=1.0)
                    # SBUF->SBUF transpose: q slab [(n,t16), w] -> chunk
                    # rows [n, t16 (reversed for bwd), w]; no DRAM bounce
                    qcap = qc[:]
                    if dr == 0:
                        dst = bass.AP(
                            tensor=qcap.tensor, offset=qcap.offset,
                            ap=[[TG * GW_, NL], [GW_, TG], [1, GW_]],
                        )
                    else:
                        dst = bass.AP(
                            tensor=qcap.tensor,
                            offset=(qcap.offset + NL * TG * GW_
                                    + (TG - 1) * GW_),
                            ap=[[TG * GW_, NL], [-GW_, TG], [1, GW_]],
                        )
                    nc.gpsimd.dma_start(out=dst, in_=qs[:])

                sc = sc_pool.tile([16, TG], f32, tag="sc")
                nc.vector.reciprocal(sc[:], qc[:, :, S_])

                for k16 in range(TG):
                    k = j * TG + k16
                    qk = qc[:, k16, 0:S_]
                    sck = sc[:, k16:k16 + 1]
                    if pending_renorm:
                        nc.vector.tensor_mul(patch[:], rinv[:], sck)
                        sck = patch[:, 0:1]
                        pending_renorm = False
                    if k == 0:
                        # A[s in {0,1}] = q * (1/acc), both chains
                        nc.vector.tensor_scalar_mul(
                            A[:, PAD:PAD + 2], qc[:, 0, 0:2], sck)
                        if use_masks:
                            nc.vector.tensor_mul(am[:, PAD:], A[:, PAD:],
                                                 msk[:, PAD:])
                        continue
                    # t1 = A + shift1(A)
                    nc.vector.tensor_add(t1[:, PAD:], A[:, PAD:],
                                         A[:, PAD - 1:PAD + S_ - 1])
                    if use_masks:
                        # t1 += shift2(masked A)
                        nc.vector.tensor_add(t1[:, PAD:], t1[:, PAD:],
                                             am[:, 0:S_])
                    else:
                        # odd states only: t1[s] += A[s-2]
                        dst_odd = t1[:, PAD + 1:PAD + S_].rearrange(
                            "p (a b) -> p a b", b=2)[:, :, 0]
                        src_odd = A[:, PAD - 1:PAD + S_ - 2].rearrange(
                            "p (a b) -> p a b", b=2)[:, :, 0]
                        nc.vector.tensor_add(dst_odd, dst_odd, src_odd)
                    # A' = (t1 * sc) * q
                    nc.vector.scalar_tensor_tensor(
                        A[:, PAD:], t1[:, PAD:], sck, qk,
                        op0=Alu.mult, op1=Alu.mult)
                    if use_masks:
                        nc.vector.tensor_mul(am[:, PAD:], A[:, PAD:],
                                             msk[:, PAD:])
                    if k in renorm_steps:
                        r = renorm_steps.index(k)
                        nc.vector.tensor_reduce(
                            Rbuf[:, r:r + 1], A[:, PAD:],
                            axis=mybir.AxisListType.X, op=Alu.max)
                        nc.vector.reciprocal(rinv[:], Rbuf[:, r:r + 1])
                        pending_renorm = True

            nc.sync.dma_start(out=res[0], in_=A[:])
            nc.sync.dma_start(out=res[1, :, 0:4], in_=Rbuf[:])
    nc.compile()
    return nc


def _get_nc(use_masks, use_renorm, dims):
    key = (use_masks, use_renorm, dims)
    if key not in _NC_CACHE:
        _NC_CACHE[key] = _build(use_masks, use_renorm, dims)
    return _NC_CACHE[key]


# ----------------------------------------------------------------------------
# device run for one full (T_, N=64, C_) problem
# ----------------------------------------------------------------------------

def _run_device(preds, t2d, dims, use_renorm):
    T_, C_, L_ = dims
    S_, GW_, NIDX_, NPAIR_, TH_, SHIFT_, renorm_steps = _derived(
        T_, C_, L_, use_renorm)

    ext = _ext_labels(t2d, S_)                    # (N, S)
    m_fwd = _skip_mask(ext)
    use_masks = bool((t2d[:, 1:] == t2d[:, :-1]).any())

    # m'[s] = m[s+2] (allowed s -> s+2); backward chain is state-reversed
    m_p = np.zeros_like(m_fwd)
    m_p[:, :-2] = m_fwd[:, 2:]
    m_bwd = m_p[:, ::-1]

    from concourse.bass_utils import run_bass_kernel_spmd

    nc = _get_nc(use_masks, use_renorm, dims)

    in_maps = []
    for c in range(NCORES):
        n0 = c * NL
        # pre-tile: (T, NL, C) -> (T/TG, NL, TG, C) with (n, t16) row order
        shard = np.ascontiguousarray(
            preds[:, n0:n0 + NL, :]
            .reshape(T_ // TG, TG, NL, C_)
            .transpose(0, 2, 1, 3))
        lists_f = np.zeros((NL, NIDX_), np.int64)
        lists_b = np.zeros((NL, NIDX_), np.int64)
        lists_f[:, :S_] = ext[n0:n0 + NL]
        lists_b[:, :S_] = ext[n0:n0 + NL, ::-1]
        im = {
            "px": shard,
            "idx_f": _wrap_idx(lists_f, NIDX_),
            "idx_b": _wrap_idx(lists_b, NIDX_),
        }
        if use_masks:
            # am-premask: am[x] = A[x] * M[x+2] so that am[s-2] carries the
            # destination mask M[s]
            mam_f = np.zeros_like(m_fwd)
            mam_f[:, :-2] = m_fwd[:, 2:]
            mam_b = np.zeros_like(m_bwd)
            mam_b[:, :-2] = m_bwd[:, 2:]
            mtile = np.zeros((16, PAD + S_), np.float32)
            mtile[0:NL, PAD:] = mam_f[n0:n0 + NL]
            mtile[NL:16, PAD:] = mam_b[n0:n0 + NL]
            im["maskd"] = mtile
        in_maps.append(im)

    out = run_bass_kernel_spmd(nc, in_maps, core_ids=list(range(NCORES)))

    # host stitch (float64): combine the two chains at the midpoint
    nr = len(renorm_steps)
    losses = np.zeros(NCORES * NL, np.float64)
    for c in range(NCORES):
        resv = out.results[c]["res"].astype(np.float64)
        for n in range(NL):
            gn = c * NL + n
            a = resv[0, n, PAD:]           # alpha_{TH-1}, natural s order
            b = resv[0, NL + n, PAD:]      # beta_{TH}, reversed s order
            mb = m_bwd[gn]
            be = b.copy()
            be[1:] += b[:-1]
            be[2:] += np.where(mb[2:], b[:-2], 0.0)
            v = float((a[::-1] * be).sum())
            rln = 0.0
            if nr:
                rln = (np.log(resv[1, n, 0:nr]).sum()
                       + np.log(resv[1, NL + n, 0:nr]).sum())
            ll = np.log(v) + rln - T_ * SHIFT_
            losses[gn] = -ll / L_
    return np.float32(losses.mean())


# ----------------------------------------------------------------------------
# entry point
# ----------------------------------------------------------------------------

def kernel(preds, targets, pred_lengths, target_lengths):
    preds = np.asarray(preds, np.float32)
    targets = np.asarray(targets, np.int32)
    pred_lengths = np.asarray(pred_lengths, np.int32)
    target_lengths = np.asarray(target_lengths, np.int32)
    t2d = targets.reshape(N, L)

    fast_ok = (
        preds.shape == (T, N, C)
        and targets.shape == (N * L,)
        and np.all(pred_lengths == T)
        and np.all(target_lengths == L)
        and np.all(targets >= 1)
        and np.all(targets < C)
        and np.isfinite(preds).all()
        and np.abs(preds).max() < 60.0
    )
    if not fast_ok:
        return _ref_numpy(preds, t2d, pred_lengths, target_lengths)

    # renorm-free DP is exact and in-range for modest logits; the
    # renormalising build remains the guard for unusual magnitudes
    use_renorm = bool(np.abs(preds).max() >= 8.0)
    return _run_device(preds, t2d, (T, C, L), use_renorm)



# revision 5
# speedup vs baseline: 1.0945x; 1.0945x over previous
"""CTC loss (nn_CTCLoss) Trainium2 Bass kernel, v2.

Sharding: data-parallel over batch N across 8 cores (8 samples/core).

Per core:
  * The shard streams through SBUF as 16 tiles of [128=(n, t16), C] f32
    (2 MB each), alternating forward tile-groups (t ascending) with
    backward tile-groups (host-stored time-reversed), in DP consumption
    order.  Per tile: GpSimd ap_gather pulls the S=2L+1 extended-label
    columns per sample (per-16-partition-group host-prepared lists),
    ScalarE exponentiates them (bias=+SHIFT), and two small SBUF->SBUF
    DMAs transpose the q slab into per-8-step chunk tiles [16, 8, S]
    (t on the free axis, fwd samples on partitions 0-7, bwd on 8-15).
  * The DP runs UNNORMALISED in the probability domain on VectorE:
    per time step A' = (A + shift1(A) + shift2_odd(A)) * q, i.e. two
    adds plus one multiply on [16, S] tiles; T/2 sequential steps per
    chain (fwd and bwd advance together on disjoint partitions).  No
    per-step softmax normalisation: a constant SHIFT keeps the chain
    inside f32 range, and the true denominators are restored on host.
  * ScalarE separately computes per-(t, n) softmax denominators with a
    fused exp+row-accumulate over each [128, C] tile (off the DP
    critical path); the [128, 16] accumulator block plus final
    alpha/beta states go back to the host, which stitches the chains at
    the midpoint in float64, applies sum(log acc) + T*SHIFT, and takes
    the batch mean.
"""

import sys

import numpy as np

for _p in ("/root/.axon_site", "/root/.axon_site/_ro/trn_rl_repo", "/opt/trn_rl_repo"):
    if _p not in sys.path:
        sys.path.append(_p)

NCORES = 8
NL = 8                   # samples per core
TG = 16                  # time steps per [128, C] tile (128 = NL * TG rows)
HG = 8                   # time steps per DP chunk tile (TG split in half)
BLANK = 0
PAD = 2                  # leading zero pad columns in DP tiles

# problem dims (the graded configuration)
T, N, C, L = 256, 64, 4096, 32

SHIFT2 = -1.0            # constant per-step scale: q = exp(x + SHIFT2)


def _derived(T_, C_, L_):
    S_ = 2 * L_ + 1
    NIDX_ = ((S_ + 15) // 16) * 16     # gather list length (%16 == 0)
    NG_ = T_ // TG                     # tile groups (fwd: 0..NG/2-1)
    NCH_ = NG_ // 2                    # DP chunks of TG steps (pairs f+b)
    TH_ = T_ // 2                      # steps per chain
    return S_, NIDX_, NG_, NCH_, TH_


# ----------------------------------------------------------------------------
# host-side helpers
# ----------------------------------------------------------------------------

def _ext_labels(t2d, S_):
    ext = np.zeros((t2d.shape[0], S_), np.int64)
    ext[:, 1::2] = t2d
    return ext


def _skip_mask(ext):
    sidx = np.arange(ext.shape[1])
    return (
        (sidx[None, :] >= 2)
        & (ext != BLANK)
        & (ext != np.roll(ext, 2, axis=1))
    )


def _ref_numpy(preds, t2d, pred_lengths, target_lengths):
    """float64 port of the reference (fallback path)."""
    preds = preds.astype(np.float64)
    Tn, n = preds.shape[0], preds.shape[1]
    S_ = 2 * t2d.shape[1] + 1
    mx = preds.max(axis=2, keepdims=True)
    lp = preds - mx - np.log(np.exp(preds - mx).sum(axis=2, keepdims=True))
    ext = _ext_labels(t2d, S_)
    lpe = lp[:, np.arange(n)[:, None], ext]
    skip_ok = _skip_mask(ext)
    NEGI = -1e30
    sidx = np.arange(S_)
    valid = sidx[None, :] < (2 * target_lengths[:, None] + 1)
    alpha = np.full((n, S_), NEGI)
    alpha[:, 0] = lpe[0, :, 0]
    alpha[:, 1] = np.where(target_lengths > 0, lpe[0, :, 1], NEGI)
    alpha = np.where(valid, alpha, NEGI)

    def lse(*a):
        m = np.maximum.reduce(a)
        m = np.where(np.isfinite(m), m, 0.0)
        return m + np.log(sum(np.exp(x - m) for x in a))

    for t in range(1, Tn):
        a2 = np.concatenate([np.full((n, 1), NEGI), alpha[:, :-1]], 1)
        a3 = np.concatenate([np.full((n, 2), NEGI), alpha[:, :-2]], 1)
        a3 = np.where(skip_ok, a3, NEGI)
        new = np.where(valid, lse(alpha, a2, a3) + lpe[t], NEGI)
        active = (t < pred_lengths)[:, None]
        alpha = np.where(active, new, alpha)
    end = 2 * target_lengths
    a_last = alpha[np.arange(n), end]
    a_prev = alpha[np.arange(n), np.maximum(end - 1, 0)]
    a_prev = np.where(target_lengths > 0, a_prev, NEGI)
    nll = -lse(a_last, a_prev)
    nll = np.where(np.isfinite(nll) & (nll < 1e29), nll, 0.0)
    return np.float32(np.mean(nll / np.maximum(target_lengths, 1)))


def _wrap_idx(lists, NIDX_):
    """lists: (NL, NIDX) int -> ap_gather wrapped layout [128, NIDX//16]."""
    out = np.zeros((128, NIDX_ // 16), np.int16)
    for g in range(NL):
        for j in range(NIDX_):
            out[g * 16 + (j % 16), j // 16] = lists[g, j]
    return out


# ----------------------------------------------------------------------------
# kernel builder
# ----------------------------------------------------------------------------

_NC_CACHE = {}


def _build(use_masks, dims):
    T_, C_, L_ = dims
    S_, NIDX_, NG_, NCH_, TH_ = _derived(T_, C_, L_)

    import concourse.bacc as bacc
    import concourse.bass as bass
    import concourse.tile as tile
    from concourse import library_config, mybir

    f32 = mybir.dt.float32
    Act = mybir.ActivationFunctionType

    nc = bacc.Bacc("TRN2", target_bir_lowering=False, debug=False)
    # tiles pre-arranged on host in DP feed order: even i = fwd group i//2,
    # odd i = bwd group NG-1-i//2 (rows time-reversed).  Each [128, C] tile
    # is 128 consecutive 16KB rows in DRAM (full HBM bandwidth).
    px = nc.dram_tensor("px", [NG_, NL, TG, C_], f32, kind="ExternalInput")
    idx_f = nc.dram_tensor("idx_f", [128, NIDX_ // 16], mybir.dt.int16,
                           kind="ExternalInput")
    idx_b = nc.dram_tensor("idx_b", [128, NIDX_ // 16], mybir.dt.int16,
                           kind="ExternalInput")
    if use_masks:
        maskd = nc.dram_tensor("maskd", [16, PAD + S_], f32,
                               kind="ExternalInput")
    res = nc.dram_tensor("res", [16, PAD + S_], f32, kind="ExternalOutput")
    accd = nc.dram_tensor("accd", [128, NG_], f32, kind="ExternalOutput")

    with tile.TileContext(nc) as tc:
        with (
            tc.tile_pool(name="mt", bufs=6) as mt_pool,
            tc.tile_pool(name="scr", bufs=2) as scr_pool,
            tc.tile_pool(name="go", bufs=4) as go_pool,
            tc.tile_pool(name="qs", bufs=4) as qs_pool,
            tc.tile_pool(name="qc", bufs=2 * NCH_) as qc_pool,
            tc.tile_pool(name="single", bufs=1) as single,
        ):
            nc.gpsimd.load_library(library_config.ap_gather)

            ixf = single.tile([128, NIDX_ // 16], mybir.dt.int16, tag="ixf")
            ixb = single.tile([128, NIDX_ // 16], mybir.dt.int16, tag="ixb")
            nc.scalar.dma_start(out=ixf[:], in_=idx_f[:])
            nc.scalar.dma_start(out=ixb[:], in_=idx_b[:])
            if use_masks:
                msk = single.tile([16, PAD + S_], f32, tag="msk")
                nc.scalar.dma_start(out=msk[:], in_=maskd[:])

            shiftb = single.tile([128, 1], f32, tag="shiftb")
            nc.vector.memset(shiftb[:], SHIFT2)

            A = single.tile([16, PAD + S_], f32, tag="A")
            t1 = single.tile([16, PAD + S_], f32, tag="t1")
            nc.vector.memset(A[:], 0.0)
            nc.vector.memset(t1[:], 0.0)
            if use_masks:
                am = single.tile([16, PAD + S_], f32, tag="am")
                nc.vector.memset(am[:], 0.0)
            accT = single.tile([128, NG_], f32, tag="accT")

            # feed order: f0, b0, f1, b1, ...  DP chunk j consumes (f_j, b_j)
            pending_bigexp = []
            qc_tiles = []

            def drain_bigexp():
                while pending_bigexp:
                    mt_, col_ = pending_bigexp.pop(0)
                    scr = scr_pool.tile([128, C_], f32, tag="scr")
                    nc.scalar.activation(scr[:], mt_, Act.Exp,
                                         bias=0.0, scale=1.0,
                                         accum_out=accT[:, col_:col_ + 1])

            for j in range(NCH_):
                for dr in range(2):
                    col = 2 * j + dr
                    mt = mt_pool.tile([128, C_], f32, tag="mt")
                    nc.sync.dma_start(
                        out=mt[:],
                        in_=px[col].rearrange("n t c -> (n t) c"))
                    go = go_pool.tile([128, NIDX_], f32, tag="go")
                    ix = ixf if dr == 0 else ixb
                    nc.gpsimd.ap_gather(go[:], mt[:], ix[:],
                                        channels=128, num_elems=C_, d=1,
                                        num_idxs=NIDX_)
                    qs = qs_pool.tile([128, S_], f32, tag="qs")
                    nc.scalar.activation(qs[:], go[:, 0:S_], Act.Exp,
                                         bias=shiftb[:, 0:1], scale=1.0)
                    # SBUF->SBUF transpose: q slab rows (n, t16) -> chunk
                    # tiles [16, HG, S]; fwd to partitions 0-7, bwd to 8-15
                    if dr == 0:
                        qc_h = []
                        for _h in range(TG // HG):
                            qct = qc_pool.tile([16, HG, S_], f32, tag="qc",
                                               name=f"qc_{j}_{_h}")
                            qc_h.append(qct)
                        qc_tiles.append(qc_h)
                    else:
                        qc_h = qc_tiles[j]
                    for h in range(TG // HG):
                        qcap = qc_h[h][:]
                        src = bass.AP(
                            tensor=qs[:].tensor,
                            offset=qs[:].offset + h * HG * S_,
                            ap=[[TG * S_, NL], [S_, HG], [1, S_]],
                        )
                        dst = bass.AP(
                            tensor=qcap.tensor,
                            offset=qcap.offset + dr * NL * HG * S_,
                            ap=[[HG * S_, NL], [S_, HG], [1, S_]],
                        )
                        nc.gpsimd.dma_start(out=dst, in_=src)
                    pending_bigexp.append((mt[:], col))

                # DP chunk j: steps k = j*TG + h*HG + k8
                for h in range(TG // HG):
                    qc = qc_tiles[j][h]
                    for k8 in range(HG):
                        k = j * TG + h * HG + k8
                        qk = qc[:, k8, 0:S_]
                        if k == 0:
                            # A[s in {0,1}] = q, both chains
                            nc.vector.tensor_copy(A[:, PAD:PAD + 2],
                                                  qc[:, 0, 0:2])
                            if use_masks:
                                nc.vector.tensor_mul(am[:, PAD:], A[:, PAD:],
                                                     msk[:, PAD:])
                            continue
                        # t1 = A + shift1(A)
                        nc.vector.tensor_add(t1[:, PAD:], A[:, PAD:],
                                             A[:, PAD - 1:PAD + S_ - 1])
                        if use_masks:
                            # t1 += shift2(masked A)
                            nc.vector.tensor_add(t1[:, PAD:], t1[:, PAD:],
                                                 am[:, 0:S_])
                        else:
                            # odd states only: t1[s] += A[s-2]
                            dst_odd = t1[:, PAD + 1:PAD + S_].rearrange(
                                "p (a b) -> p a b", b=2)[:, :, 0]
                            src_odd = A[:, PAD - 1:PAD + S_ - 2].rearrange(
                                "p (a b) -> p a b", b=2)[:, :, 0]
                            nc.vector.tensor_add(dst_odd, dst_odd, src_odd)
                        # A' = t1 * q
                        nc.vector.tensor_mul(A[:, PAD:], t1[:, PAD:], qk)
                        if use_masks:
                            nc.vector.tensor_mul(am[:, PAD:], A[:, PAD:],
                                                 msk[:, PAD:])
                    # big-exp (softmax denominators) trails the DP feed so
                    # ScalarE never delays the next chunk's small exp
                    if h == 0:
                        drain_bigexp()
            drain_bigexp()

            nc.sync.dma_start(out=accd[:], in_=accT[:])
            nc.sync.dma_start(out=res[:], in_=A[:])
    nc.compile()
    return nc


def _get_nc(use_masks, dims):
    key = (use_masks, dims)
    if key not in _NC_CACHE:
        _NC_CACHE[key] = _build(use_masks, dims)
    return _NC_CACHE[key]


# ----------------------------------------------------------------------------
# device run for one full (T_, N=64, C_) problem
# ----------------------------------------------------------------------------

def _run_device(preds, t2d, dims):
    T_, C_, L_ = dims
    S_, NIDX_, NG_, NCH_, TH_ = _derived(T_, C_, L_)

    ext = _ext_labels(t2d, S_)                    # (N, S)
    m_fwd = _skip_mask(ext)
    use_masks = bool((t2d[:, 1:] == t2d[:, :-1]).any())

    # m'[s] = m[s+2] (allowed s -> s+2); backward chain is state-reversed
    m_p = np.zeros_like(m_fwd)
    m_p[:, :-2] = m_fwd[:, 2:]
    m_bwd = m_p[:, ::-1]

    from concourse.bass_utils import run_bass_kernel_spmd

    nc = _get_nc(use_masks, dims)

    in_maps = []
    for c in range(NCORES):
        n0 = c * NL
        # pre-tile into DP feed order: even tile i: fwd group i//2 rows
        # (n, t16) with t = 16*(i//2) + t16; odd tile i: bwd group
        # NG-1-i//2 with t = T-1 - (16*(i//2) + t16)  (time-reversed)
        sh = preds[:, n0:n0 + NL, :]               # (T, NL, C)
        tiles = np.empty((NG_, NL, TG, C_), np.float32)
        for j in range(NCH_):
            tf = sh[16 * j:16 * j + TG]            # (TG, NL, C)
            tiles[2 * j] = tf.transpose(1, 0, 2)
            tb = sh[T_ - 1 - 16 * j - (TG - 1):T_ - 16 * j][::-1]
            tiles[2 * j + 1] = tb.transpose(1, 0, 2)
        lists_f = np.zeros((NL, NIDX_), np.int64)
        lists_b = np.zeros((NL, NIDX_), np.int64)
        lists_f[:, :S_] = ext[n0:n0 + NL]
        lists_b[:, :S_] = ext[n0:n0 + NL, ::-1]
        im = {
            "px": np.ascontiguousarray(tiles),
            "idx_f": _wrap_idx(lists_f, NIDX_),
            "idx_b": _wrap_idx(lists_b, NIDX_),
        }
        if use_masks:
            # am-premask: am[x] = A[x] * M[x+2] so that am[s-2] carries the
            # destination mask M[s]
            mam_f = np.zeros_like(m_fwd)
            mam_f[:, :-2] = m_fwd[:, 2:]
            mam_b = np.zeros_like(m_bwd)
            mam_b[:, :-2] = m_bwd[:, 2:]
            mtile = np.zeros((16, PAD + S_), np.float32)
            mtile[0:NL, PAD:] = mam_f[n0:n0 + NL]
            mtile[NL:16, PAD:] = mam_b[n0:n0 + NL]
            im["maskd"] = mtile
        in_maps.append(im)

    out = run_bass_kernel_spmd(nc, in_maps, core_ids=list(range(NCORES)))

    # host stitch (float64): combine the two chains at the midpoint and
    # restore the softmax denominators: ll = ln v - sum_t ln acc - T*SHIFT
    losses = np.zeros(NCORES * NL, np.float64)
    for c in range(NCORES):
        resv = out.results[c]["res"].astype(np.float64)
        accv = out.results[c]["accd"].astype(np.float64)  # [128, NG]
        if not (np.isfinite(accv).all() and (accv > 0).all()):
            return None
        lacc = np.log(accv).reshape(NL, TG, NG_).sum(axis=(1, 2))  # per n
        for n in range(NL):
            gn = c * NL + n
            a = resv[n, PAD:]             # alpha_{TH-1}, natural s order
            b = resv[NL + n, PAD:]        # beta_{TH}, reversed s order
            mb = m_bwd[gn]
            be = b.copy()
            be[1:] += b[:-1]
            be[2:] += np.where(mb[2:], b[:-2], 0.0)
            v = float((a[::-1] * be).sum())
            if not (np.isfinite(v) and v > 0.0):
                return None
            ll = np.log(v) - lacc[n] - T_ * SHIFT2
            losses[gn] = -ll / L_
    return np.float32(losses.mean())


# ----------------------------------------------------------------------------
# entry point
# ----------------------------------------------------------------------------

def kernel(preds, targets, pred_lengths, target_lengths):
    preds = np.asarray(preds, np.float32)
    targets = np.asarray(targets, np.int32)
    pred_lengths = np.asarray(pred_lengths, np.int32)
    target_lengths = np.asarray(target_lengths, np.int32)
    t2d = targets.reshape(N, L)

    fast_ok = (
        preds.shape == (T, N, C)
        and targets.shape == (N * L,)
        and np.all(pred_lengths == T)
        and np.all(target_lengths == L)
        and np.all(targets >= 1)
        and np.all(targets < C)
        and np.isfinite(preds).all()
        and np.abs(preds).max() < 8.0
    )
    if fast_ok:
        r = _run_device(preds, t2d, (T, C, L))
        if r is not None:
            return r
    return _ref_numpy(preds, t2d, pred_lengths, target_lengths)


# revision 6
# speedup vs baseline: 1.1974x; 1.0940x over previous
"""CTC loss (nn_CTCLoss) Trainium2 Bass kernel, v3.

Sharding: data-parallel over batch N across 8 cores (8 samples/core).

Per core, two decoupled device pipelines:
  * DP feed: the host pre-gathers the S=2L+1 extended-label logits per
    (chain, sample, step) into a slab qx [16, (T/2)*S] f32 (row p<8 =
    forward chain of sample p, row p>=8 = backward chain, time- and
    state-reversed so both chains read forward).  One small early DMA
    brings it in; ScalarE exponentiates it per 16-step block
    (q = exp(x + SHIFT), bf16 out); VectorE runs the UNNORMALISED DP in
    the probability domain: per step A' = (A + shift1(A) + shift2_odd(A))
    * q -- two adds plus one multiply on [16, S] bf16 tiles, T/2
    sequential steps, both chains advancing together on disjoint
    partitions.  No per-step softmax normalisation: a constant SHIFT
    keeps the chain inside range; denominators are restored on host.
  * Denominators: the (T, NL, C) shard streams through SBUF as 16
    [128, C] f32 tiles (one 2 MB contiguous DMA each); ScalarE computes
    exp() with a fused per-row accumulate, collecting the softmax
    denominator of every (t, n) into accT [128, 16].

Final alpha/beta states (bf16) plus accT go back to the host, which
stitches the chains at the midpoint in float64 and applies
sum(log acc) + T*SHIFT before the batch mean.
"""

import sys

import numpy as np

for _p in ("/root/.axon_site", "/root/.axon_site/_ro/trn_rl_repo", "/opt/trn_rl_repo"):
    if _p not in sys.path:
        sys.path.append(_p)

NCORES = 8
NL = 8                   # samples per core
TG = 16                  # time steps per [128, C] tile and per q block
BLANK = 0
PAD = 2                  # leading zero pad columns in DP tiles

# problem dims (the graded configuration)
T, N, C, L = 256, 64, 4096, 32

SHIFT2 = -1.0            # constant per-step scale: q = exp(x + SHIFT2)


def _derived(T_, C_, L_):
    S_ = 2 * L_ + 1
    NG_ = T_ // TG                     # [128, C] tile groups
    TH_ = T_ // 2                      # steps per chain
    NB_ = TH_ // TG                    # q blocks
    return S_, NG_, TH_, NB_


# ----------------------------------------------------------------------------
# host-side helpers
# ----------------------------------------------------------------------------

def _ext_labels(t2d, S_):
    ext = np.zeros((t2d.shape[0], S_), np.int64)
    ext[:, 1::2] = t2d
    return ext


def _skip_mask(ext):
    sidx = np.arange(ext.shape[1])
    return (
        (sidx[None, :] >= 2)
        & (ext != BLANK)
        & (ext != np.roll(ext, 2, axis=1))
    )


def _ref_numpy(preds, t2d, pred_lengths, target_lengths):
    """float64 port of the reference (fallback path)."""
    preds = preds.astype(np.float64)
    Tn, n = preds.shape[0], preds.shape[1]
    S_ = 2 * t2d.shape[1] + 1
    mx = preds.max(axis=2, keepdims=True)
    lp = preds - mx - np.log(np.exp(preds - mx).sum(axis=2, keepdims=True))
    ext = _ext_labels(t2d, S_)
    lpe = lp[:, np.arange(n)[:, None], ext]
    skip_ok = _skip_mask(ext)
    NEGI = -1e30
    sidx = np.arange(S_)
    valid = sidx[None, :] < (2 * target_lengths[:, None] + 1)
    alpha = np.full((n, S_), NEGI)
    alpha[:, 0] = lpe[0, :, 0]
    alpha[:, 1] = np.where(target_lengths > 0, lpe[0, :, 1], NEGI)
    alpha = np.where(valid, alpha, NEGI)

    def lse(*a):
        m = np.maximum.reduce(a)
        m = np.where(np.isfinite(m), m, 0.0)
        return m + np.log(sum(np.exp(x - m) for x in a))

    for t in range(1, Tn):
        a2 = np.concatenate([np.full((n, 1), NEGI), alpha[:, :-1]], 1)
        a3 = np.concatenate([np.full((n, 2), NEGI), alpha[:, :-2]], 1)
        a3 = np.where(skip_ok, a3, NEGI)
        new = np.where(valid, lse(alpha, a2, a3) + lpe[t], NEGI)
        active = (t < pred_lengths)[:, None]
        alpha = np.where(active, new, alpha)
    end = 2 * target_lengths
    a_last = alpha[np.arange(n), end]
    a_prev = alpha[np.arange(n), np.maximum(end - 1, 0)]
    a_prev = np.where(target_lengths > 0, a_prev, NEGI)
    nll = -lse(a_last, a_prev)
    nll = np.where(np.isfinite(nll) & (nll < 1e29), nll, 0.0)
    return np.float32(np.mean(nll / np.maximum(target_lengths, 1)))


# ----------------------------------------------------------------------------
# kernel builder
# ----------------------------------------------------------------------------

_NC_CACHE = {}


def _build(use_masks, use_bf16, dims):
    T_, C_, L_ = dims
    S_, NG_, TH_, NB_ = _derived(T_, C_, L_)

    import concourse.bacc as bacc
    import concourse.tile as tile
    from concourse import library_config, mybir

    f32 = mybir.dt.float32
    dpt = mybir.dt.bfloat16 if use_bf16 else f32
    Act = mybir.ActivationFunctionType

    nc = bacc.Bacc("TRN2", target_bir_lowering=False, debug=False)
    # preds shard pre-tiled on host: [group, n, t16, c]; each [128, C] tile
    # load reads 128 consecutive 16KB rows (full HBM bandwidth)
    px = nc.dram_tensor("px", [NG_, NL, TG, C_], f32, kind="ExternalInput")
    # host-gathered extended-label logits, DP order (see module docstring)
    qx = nc.dram_tensor("qx", [16, TH_ * S_], f32, kind="ExternalInput")
    if use_masks:
        maskd = nc.dram_tensor("maskd", [16, PAD + S_], dpt,
                               kind="ExternalInput")
    res = nc.dram_tensor("res", [16, PAD + S_], dpt, kind="ExternalOutput")
    accd = nc.dram_tensor("accd", [128, NG_], f32, kind="ExternalOutput")

    with tile.TileContext(nc) as tc:
        with (
            tc.tile_pool(name="mt", bufs=5) as mt_pool,
            tc.tile_pool(name="scr", bufs=1) as scr_pool,
            tc.tile_pool(name="single", bufs=1) as single,
        ):
            qxt = single.tile([16, TH_ * S_], f32, tag="qxt")
            nc.scalar.dma_start(out=qxt[:], in_=qx[:])
            if use_masks:
                msk = single.tile([16, PAD + S_], dpt, tag="msk")
                nc.scalar.dma_start(out=msk[:], in_=maskd[:])

            shiftb = single.tile([16, 1], f32, tag="shiftb")
            nc.vector.memset(shiftb[:], SHIFT2)

            A = single.tile([16, PAD + S_], dpt, tag="A")
            t1 = single.tile([16, PAD + S_], dpt, tag="t1")
            nc.vector.memset(A[:], 0.0)
            nc.vector.memset(t1[:], 0.0)
            if use_masks:
                am = single.tile([16, PAD + S_], dpt, tag="am")
                nc.vector.memset(am[:], 0.0)
            accT = single.tile([128, NG_], f32, tag="accT")

            # q blocks: exp of the host-gathered slab, all issued first so
            # ScalarE fully feeds the DP before the big exps queue behind
            qct = []
            for j in range(NB_):
                qcj = single.tile([16, TG * S_], dpt, tag=f"qc{j}",
                                  name=f"qc_{j}")
                nc.scalar.activation(qcj[:], qxt[:, j * TG * S_:(j + 1) * TG * S_],
                                     Act.Exp, bias=shiftb[:, 0:1], scale=1.0)
                qct.append(qcj)

            # softmax denominators: stream the shard, fused exp+row-sum
            for g in range(NG_):
                mt = mt_pool.tile([128, C_], f32, tag="mt")
                nc.sync.dma_start(out=mt[:],
                                  in_=px[g].rearrange("n t c -> (n t) c"))
                scr = scr_pool.tile([128, C_], f32, tag="scr")
                nc.scalar.activation(scr[:], mt[:], Act.Exp,
                                     bias=0.0, scale=1.0,
                                     accum_out=accT[:, g:g + 1])

            # DP: T/2 sequential steps, fwd chain on partitions 0-7 and
            # (time/state-reversed) bwd chain on 8-15 advancing together
            for k in range(TH_):
                qc = qct[k // TG]
                o = (k % TG) * S_
                qk = qc[:, o:o + S_]
                if k == 0:
                    # A[s in {0,1}] = q, both chains
                    nc.vector.tensor_copy(A[:, PAD:PAD + 2], qc[:, 0:2])
                    if use_masks:
                        nc.vector.tensor_mul(am[:, PAD:], A[:, PAD:],
                                             msk[:, PAD:])
                    continue
                # t1 = A + shift1(A)
                nc.vector.tensor_add(t1[:, PAD:], A[:, PAD:],
                                     A[:, PAD - 1:PAD + S_ - 1])
                if use_masks:
                    # t1 += shift2(masked A)
                    nc.vector.tensor_add(t1[:, PAD:], t1[:, PAD:],
                                         am[:, 0:S_])
                else:
                    # odd states only: t1[s] += A[s-2]
                    dst_odd = t1[:, PAD + 1:PAD + S_].rearrange(
                        "p (a b) -> p a b", b=2)[:, :, 0]
                    src_odd = A[:, PAD - 1:PAD + S_ - 2].rearrange(
                        "p (a b) -> p a b", b=2)[:, :, 0]
                    nc.vector.tensor_add(dst_odd, dst_odd, src_odd)
                # A' = t1 * q
                nc.vector.tensor_mul(A[:, PAD:], t1[:, PAD:], qk)
                if use_masks:
                    nc.vector.tensor_mul(am[:, PAD:], A[:, PAD:],
                                         msk[:, PAD:])

            nc.scalar.dma_start(out=accd[:], in_=accT[:])
            nc.sync.dma_start(out=res[:], in_=A[:])
    nc.compile()
    return nc


def _get_nc(use_masks, use_bf16, dims):
    key = (use_masks, use_bf16, dims)
    if key not in _NC_CACHE:
        _NC_CACHE[key] = _build(use_masks, use_bf16, dims)
    return _NC_CACHE[key]


# ----------------------------------------------------------------------------
# device run for one full (T_, N=64, C_) problem
# ----------------------------------------------------------------------------

def _run_device(preds, t2d, dims, use_bf16=True):
    T_, C_, L_ = dims
    S_, NG_, TH_, NB_ = _derived(T_, C_, L_)

    ext = _ext_labels(t2d, S_)                    # (N, S)
    m_fwd = _skip_mask(ext)
    use_masks = bool((t2d[:, 1:] == t2d[:, :-1]).any())

    # m'[s] = m[s+2] (allowed s -> s+2); backward chain is state-reversed
    m_p = np.zeros_like(m_fwd)
    m_p[:, :-2] = m_fwd[:, 2:]
    m_bwd = m_p[:, ::-1]

    from concourse.bass_utils import run_bass_kernel_spmd

    nc = _get_nc(use_masks, use_bf16, dims)

    in_maps = []
    for c in range(NCORES):
        n0 = c * NL
        sh = preds[:, n0:n0 + NL, :]               # (T, NL, C)
        # pre-tile: (T, NL, C) -> (NG, NL, TG, C) with (n, t16) row order
        tiles = np.ascontiguousarray(
            sh.reshape(NG_, TG, NL, C_).transpose(0, 2, 1, 3))
        # host-gathered DP slab [16, TH*S]:
        #   row n   (fwd): col k*S+s = preds[k,       n, ext[n, s]]
        #   row 8+n (bwd): col k*S+s = preds[T-1-k, n, ext[n, S-1-s]]
        nidx = np.arange(NL)[:, None, None]
        kidx = np.arange(TH_)[None, :, None]
        qf = sh[kidx, nidx, ext[n0:n0 + NL, None, :]]            # (NL,TH,S)
        qb = sh[T_ - 1 - kidx, nidx, ext[n0:n0 + NL, None, ::-1]]
        qxa = np.empty((16, TH_ * S_), np.float32)
        qxa[0:NL] = qf.reshape(NL, TH_ * S_)
        qxa[NL:16] = qb.reshape(NL, TH_ * S_)
        im = {"px": tiles, "qx": qxa}
        if use_masks:
            # am-premask: am[x] = A[x] * M[x+2] so that am[s-2] carries the
            # destination mask M[s]
            mam_f = np.zeros_like(m_fwd)
            mam_f[:, :-2] = m_fwd[:, 2:]
            mam_b = np.zeros_like(m_bwd)
            mam_b[:, :-2] = m_bwd[:, 2:]
            mtile = np.zeros((16, PAD + S_), np.float32)
            mtile[0:NL, PAD:] = mam_f[n0:n0 + NL]
            mtile[NL:16, PAD:] = mam_b[n0:n0 + NL]
            if use_bf16:
                import ml_dtypes
                mtile = mtile.astype(ml_dtypes.bfloat16)
            im["maskd"] = mtile
        in_maps.append(im)

    out = run_bass_kernel_spmd(nc, in_maps, core_ids=list(range(NCORES)))

    # host stitch (float64): combine the two chains at the midpoint and
    # restore the softmax denominators: ll = ln v - sum_t ln acc - T*SHIFT
    losses = np.zeros(NCORES * NL, np.float64)
    for c in range(NCORES):
        resv = np.asarray(out.results[c]["res"]).astype(np.float64)
        accv = np.asarray(out.results[c]["accd"]).astype(np.float64)
        if not (np.isfinite(accv).all() and (accv > 0).all()):
            return None
        lacc = np.log(accv).reshape(NL, TG, NG_).sum(axis=(1, 2))  # per n
        for n in range(NL):
            gn = c * NL + n
            a = resv[n, PAD:]             # alpha_{TH-1}, natural s order
            b = resv[NL + n, PAD:]        # beta_{TH}, reversed s order
            mb = m_bwd[gn]
            be = b.copy()
            be[1:] += b[:-1]
            be[2:] += np.where(mb[2:], b[:-2], 0.0)
            v = float((a[::-1] * be).sum())
            if not (np.isfinite(v) and v > 0.0):
                return None
            ll = np.log(v) - lacc[n] - T_ * SHIFT2
            losses[gn] = -ll / L_
    return np.float32(losses.mean())


# ----------------------------------------------------------------------------
# entry point
# ----------------------------------------------------------------------------

def kernel(preds, targets, pred_lengths, target_lengths):
    preds = np.asarray(preds, np.float32)
    targets = np.asarray(targets, np.int32)
    pred_lengths = np.asarray(pred_lengths, np.int32)
    target_lengths = np.asarray(target_lengths, np.int32)
    t2d = targets.reshape(N, L)

    fast_ok = (
        preds.shape == (T, N, C)
        and targets.shape == (N * L,)
        and np.all(pred_lengths == T)
        and np.all(target_lengths == L)
        and np.all(targets >= 1)
        and np.all(targets < C)
        and np.isfinite(preds).all()
        and np.abs(preds).max() < 8.0
    )
    if fast_ok:
        r = _run_device(preds, t2d, (T, C, L))
        if r is not None:
            return r
    return _ref_numpy(preds, t2d, pred_lengths, target_lengths)


# revision 7
# speedup vs baseline: 1.3177x; 1.1005x over previous
"""CTC loss (nn_CTCLoss) Trainium2 Bass kernel, v4.

Sharding: data-parallel over batch N across 8 cores (8 samples/core).

Per core, two decoupled device pipelines:
  * DP feed: the host pre-gathers the S=2L+1 extended-label logits per
    (chain, sample, step) into a slab qx [16, (T/2)*S] bf16 (row p<8 =
    forward chain of sample p, row p>=8 = backward chain, time- and
    state-reversed so both chains read forward).  States are stored
    de-interleaved per step: [labels (L), blanks (L+1)], which turns the
    CTC three-tap recurrence into contiguous short ops.  Two early DMAs
    (first 16-step block, then the rest) bring it in; ScalarE
    exponentiates per 16-step block (q = exp(x + SHIFT), f32 out);
    VectorE runs the UNNORMALISED DP in the probability domain:
        u  = B + shift(Lb)        [L+1 wide]   (blank update pre-mul)
        w  = u + Lb               [L wide]     (label update pre-mul)
        A' = [w | u] * q          [S wide]
    i.e. 3 short ops per time step, T/2 sequential steps, both chains
    advancing together on disjoint partitions.  No per-step softmax
    normalisation: a constant SHIFT keeps the chain inside f32 range and
    the true denominators are restored on host.
  * Denominators: the (T, NL, C) shard streams through SBUF as 16
    [128, C] f32 tiles (one 2 MB contiguous DMA each); ScalarE computes
    exp() with a fused per-row accumulate, collecting the softmax
    denominator of every (t, n) into accT [128, 16].

Final alpha/beta states plus accT go back to the host, which stitches
the chains at the midpoint in float64 and applies sum(log acc) +
T*SHIFT before the batch mean.
"""

import sys

import numpy as np

for _p in ("/root/.axon_site", "/root/.axon_site/_ro/trn_rl_repo", "/opt/trn_rl_repo"):
    if _p not in sys.path:
        sys.path.append(_p)

NCORES = 8
NL = 8                   # samples per core
TG = 16                  # time steps per [128, C] tile and per q block
BLANK = 0
PAD = 2                  # leading zero pad columns in DP tiles

# problem dims (the graded configuration)
T, N, C, L = 256, 64, 4096, 32

SHIFT2 = -1.0            # constant per-step scale: q = exp(x + SHIFT2)


def _derived(T_, C_, L_):
    S_ = 2 * L_ + 1
    NG_ = T_ // TG                     # [128, C] tile groups
    TH_ = T_ // 2                      # steps per chain
    NB_ = TH_ // TG                    # q blocks
    return S_, NG_, TH_, NB_


# ----------------------------------------------------------------------------
# host-side helpers
# ----------------------------------------------------------------------------

def _ext_labels(t2d, S_):
    ext = np.zeros((t2d.shape[0], S_), np.int64)
    ext[:, 1::2] = t2d
    return ext


def _skip_mask(ext):
    sidx = np.arange(ext.shape[1])
    return (
        (sidx[None, :] >= 2)
        & (ext != BLANK)
        & (ext != np.roll(ext, 2, axis=1))
    )


def _ref_numpy(preds, t2d, pred_lengths, target_lengths):
    """float64 port of the reference (fallback path)."""
    preds = preds.astype(np.float64)
    Tn, n = preds.shape[0], preds.shape[1]
    S_ = 2 * t2d.shape[1] + 1
    mx = preds.max(axis=2, keepdims=True)
    lp = preds - mx - np.log(np.exp(preds - mx).sum(axis=2, keepdims=True))
    ext = _ext_labels(t2d, S_)
    lpe = lp[:, np.arange(n)[:, None], ext]
    skip_ok = _skip_mask(ext)
    NEGI = -1e30
    sidx = np.arange(S_)
    valid = sidx[None, :] < (2 * target_lengths[:, None] + 1)
    alpha = np.full((n, S_), NEGI)
    alpha[:, 0] = lpe[0, :, 0]
    alpha[:, 1] = np.where(target_lengths > 0, lpe[0, :, 1], NEGI)
    alpha = np.where(valid, alpha, NEGI)

    def lse(*a):
        m = np.maximum.reduce(a)
        m = np.where(np.isfinite(m), m, 0.0)
        return m + np.log(sum(np.exp(x - m) for x in a))

    for t in range(1, Tn):
        a2 = np.concatenate([np.full((n, 1), NEGI), alpha[:, :-1]], 1)
        a3 = np.concatenate([np.full((n, 2), NEGI), alpha[:, :-2]], 1)
        a3 = np.where(skip_ok, a3, NEGI)
        new = np.where(valid, lse(alpha, a2, a3) + lpe[t], NEGI)
        active = (t < pred_lengths)[:, None]
        alpha = np.where(active, new, alpha)
    end = 2 * target_lengths
    a_last = alpha[np.arange(n), end]
    a_prev = alpha[np.arange(n), np.maximum(end - 1, 0)]
    a_prev = np.where(target_lengths > 0, a_prev, NEGI)
    nll = -lse(a_last, a_prev)
    nll = np.where(np.isfinite(nll) & (nll < 1e29), nll, 0.0)
    return np.float32(np.mean(nll / np.maximum(target_lengths, 1)))


# ----------------------------------------------------------------------------
# kernel builder
# ----------------------------------------------------------------------------

_NC_CACHE = {}


def _build(use_masks, dims):
    """use_masks=False: de-interleaved [Lb | B] DP layout (no repeated
    labels).  use_masks=True: natural state order with mask tiles."""
    T_, C_, L_ = dims
    S_, NG_, TH_, NB_ = _derived(T_, C_, L_)
    PL = PAD + L_            # start of the blank block (no-repeat layout)

    import concourse.bacc as bacc
    import concourse.tile as tile
    from concourse import mybir

    f32 = mybir.dt.float32
    bf16 = mybir.dt.bfloat16
    Act = mybir.ActivationFunctionType

    nc = bacc.Bacc("TRN2", target_bir_lowering=False, debug=False)
    # preds shard pre-tiled on host: [group, n, t16, c]; each [128, C] tile
    # load reads 128 consecutive 16KB rows (full HBM bandwidth)
    px = nc.dram_tensor("px", [NG_, NL, TG, C_], f32, kind="ExternalInput")
    # host-gathered extended-label logits in DP order (see module docstring)
    qx = nc.dram_tensor("qx", [16, TH_ * S_], bf16, kind="ExternalInput")
    if use_masks:
        maskd = nc.dram_tensor("maskd", [16, PAD + S_], f32,
                               kind="ExternalInput")
    res = nc.dram_tensor("res", [16, PAD + S_], f32, kind="ExternalOutput")
    accd = nc.dram_tensor("accd", [128, NG_], f32, kind="ExternalOutput")

    with tile.TileContext(nc) as tc:
        with (
            tc.tile_pool(name="mt", bufs=7) as mt_pool,
            tc.tile_pool(name="scr", bufs=1) as scr_pool,
            tc.tile_pool(name="single", bufs=1) as single,
        ):
            with tc.high_priority():
                # DP feed: first block lands fast, the rest right behind
                qxt0 = single.tile([16, TG * S_], bf16, tag="qxt0")
                nc.scalar.dma_start(out=qxt0[:], in_=qx[:, 0:TG * S_])
                qxtr = single.tile([16, (TH_ - TG) * S_], bf16, tag="qxtr")
                nc.scalar.dma_start(out=qxtr[:], in_=qx[:, TG * S_:])
                if use_masks:
                    msk = single.tile([16, PAD + S_], f32, tag="msk")
                    nc.scalar.dma_start(out=msk[:], in_=maskd[:])

                shiftb = single.tile([16, 1], f32, tag="shiftb")
                nc.vector.memset(shiftb[:], SHIFT2)

                A = single.tile([16, PAD + S_], f32, tag="A")
                t1 = single.tile([16, PAD + S_], f32, tag="t1")
                nc.vector.memset(A[:], 0.0)
                nc.vector.memset(t1[:], 0.0)
                if use_masks:
                    am = single.tile([16, PAD + S_], f32, tag="am")
                    nc.vector.memset(am[:], 0.0)

                # q blocks: exp of the host-gathered slab
                qct = []
                for j in range(NB_):
                    qcj = single.tile([16, TG * S_], f32, tag=f"qc{j}",
                                      name=f"qc_{j}")
                    if j == 0:
                        src = qxt0[:]
                    else:
                        src = qxtr[:, (j - 1) * TG * S_:j * TG * S_]
                    nc.scalar.activation(qcj[:], src, Act.Exp,
                                         bias=shiftb[:, 0:1], scale=1.0)
                    qct.append(qcj)

            accT = single.tile([128, NG_], f32, tag="accT")

            # softmax denominators: stream the shard, fused exp+row-sum
            for g in range(NG_):
                mt = mt_pool.tile([128, C_], f32, tag="mt")
                nc.sync.dma_start(out=mt[:],
                                  in_=px[g].rearrange("n t c -> (n t) c"))
                scr = scr_pool.tile([128, C_], bf16, tag="scr")
                nc.scalar.activation(scr[:], mt[:], Act.Exp,
                                     bias=0.0, scale=1.0,
                                     accum_out=accT[:, g:g + 1])

            # DP: T/2 sequential steps, fwd chain on partitions 0-7 and
            # (time/state-reversed) bwd chain on 8-15 advancing together
            for k in range(TH_):
                qc = qct[k // TG]
                o = (k % TG) * S_
                qk = qc[:, o:o + S_]
                if k == 0:
                    if use_masks:
                        nc.vector.tensor_copy(A[:, PAD:PAD + 2], qc[:, 0:2])
                        nc.vector.tensor_mul(am[:, PAD:], A[:, PAD:],
                                             msk[:, PAD:])
                    else:
                        # A[Lb 0] = q[l0], A[B 0] = q[b0] (cols o, o+L)
                        dst = A[:, PAD:PAD + 2 * L_].rearrange(
                            "p (a b) -> p a b", b=L_)[:, :, 0]
                        src = qc[:, o:o + 2 * L_].rearrange(
                            "p (a b) -> p a b", b=L_)[:, :, 0]
                        nc.vector.tensor_copy(dst, src)
                    continue
                if use_masks:
                    # t1 = A + shift1(A); t1 += shift2(masked A); A' = t1*q
                    nc.vector.tensor_add(t1[:, PAD:], A[:, PAD:],
                                         A[:, PAD - 1:PAD + S_ - 1])
                    nc.vector.tensor_add(t1[:, PAD:], t1[:, PAD:],
                                         am[:, 0:S_])
                    nc.vector.tensor_mul(A[:, PAD:], t1[:, PAD:], qk)
                    nc.vector.tensor_mul(am[:, PAD:], A[:, PAD:],
                                         msk[:, PAD:])
                else:
                    # u = B + shift(Lb)  (blank pre-mul, L+1 wide)
                    nc.vector.tensor_add(t1[:, PL:PL + L_ + 1],
                                         A[:, PL:PL + L_ + 1],
                                         A[:, PAD - 1:PAD + L_])
                    # w = u + Lb  (label pre-mul, L wide)
                    nc.vector.tensor_add(t1[:, PAD:PAD + L_],
                                         t1[:, PL:PL + L_],
                                         A[:, PAD:PAD + L_])
                    # A' = [w | u] * q
                    nc.vector.tensor_mul(A[:, PAD:PAD + S_],
                                         t1[:, PAD:PAD + S_], qk)

            nc.scalar.dma_start(out=accd[:], in_=accT[:])
            nc.sync.dma_start(out=res[:], in_=A[:])
    nc.compile()
    return nc


def _get_nc(use_masks, dims):
    key = (use_masks, dims)
    if key not in _NC_CACHE:
        _NC_CACHE[key] = _build(use_masks, dims)
    return _NC_CACHE[key]


# ----------------------------------------------------------------------------
# device run for one full (T_, N=64, C_) problem
# ----------------------------------------------------------------------------

def _run_device(preds, t2d, dims):
    T_, C_, L_ = dims
    S_, NG_, TH_, NB_ = _derived(T_, C_, L_)
    import ml_dtypes

    ext = _ext_labels(t2d, S_)                    # (N, S)
    m_fwd = _skip_mask(ext)
    use_masks = bool((t2d[:, 1:] == t2d[:, :-1]).any())

    # m'[s] = m[s+2] (allowed s -> s+2); backward chain is state-reversed
    m_p = np.zeros_like(m_fwd)
    m_p[:, :-2] = m_fwd[:, 2:]
    m_bwd = m_p[:, ::-1]

    from concourse.bass_utils import run_bass_kernel_spmd

    nc = _get_nc(use_masks, dims)

    # gather column orders: natural (masks) or [labels | blanks] split
    if use_masks:
        cols_f = ext                               # (N, S)
        cols_b = ext[:, ::-1]
    else:
        cols_f = np.concatenate([ext[:, 1::2], ext[:, 0::2]], axis=1)
        extr = ext[:, ::-1]
        cols_b = np.concatenate([extr[:, 1::2], extr[:, 0::2]], axis=1)

    in_maps = []
    for c in range(NCORES):
        n0 = c * NL
        sh = preds[:, n0:n0 + NL, :]               # (T, NL, C)
        # pre-tile: (T, NL, C) -> (NG, NL, TG, C) with (n, t16) row order
        tiles = np.ascontiguousarray(
            sh.reshape(NG_, TG, NL, C_).transpose(0, 2, 1, 3))
        # host-gathered DP slab [16, TH*S]
        nidx = np.arange(NL)[:, None, None]
        kidx = np.arange(TH_)[None, :, None]
        qf = sh[kidx, nidx, cols_f[n0:n0 + NL, None, :]]         # (NL,TH,S)
        qb = sh[T_ - 1 - kidx, nidx, cols_b[n0:n0 + NL, None, :]]
        qxa = np.empty((16, TH_ * S_), np.float32)
        qxa[0:NL] = qf.reshape(NL, TH_ * S_)
        qxa[NL:16] = qb.reshape(NL, TH_ * S_)
        im = {"px": tiles, "qx": qxa.astype(ml_dtypes.bfloat16)}
        if use_masks:
            # am-premask: am[x] = A[x] * M[x+2] so that am[s-2] carries the
            # destination mask M[s]
            mam_f = np.zeros_like(m_fwd)
            mam_f[:, :-2] = m_fwd[:, 2:]
            mam_b = np.zeros_like(m_bwd)
            mam_b[:, :-2] = m_bwd[:, 2:]
            mtile = np.zeros((16, PAD + S_), np.float32)
            mtile[0:NL, PAD:] = mam_f[n0:n0 + NL]
            mtile[NL:16, PAD:] = mam_b[n0:n0 + NL]
            im["maskd"] = mtile
        in_maps.append(im)

    out = run_bass_kernel_spmd(nc, in_maps, core_ids=list(range(NCORES)))

    # host stitch (float64): combine the two chains at the midpoint and
    # restore the softmax denominators: ll = ln v - sum_t ln acc - T*SHIFT
    losses = np.zeros(NCORES * NL, np.float64)
    for c in range(NCORES):
        resv = np.asarray(out.results[c]["res"]).astype(np.float64)
        accv = np.asarray(out.results[c]["accd"]).astype(np.float64)
        if not (np.isfinite(accv).all() and (accv > 0).all()):
            return None
        lacc = np.log(accv).reshape(NL, TG, NG_).sum(axis=(1, 2))  # per n
        for n in range(NL):
            gn = c * NL + n
            ar = resv[n, PAD:]            # alpha_{TH-1}
            br = resv[NL + n, PAD:]       # beta_{TH}, reversed s order
            if use_masks:
                a, b = ar, br
            else:
                # de-interleave [labels | blanks] back to natural order
                a = np.empty(S_)
                a[1::2] = ar[0:L_]
                a[0::2] = ar[L_:S_]
                b = np.empty(S_)
                b[1::2] = br[0:L_]
                b[0::2] = br[L_:S_]
            mb = m_bwd[gn]
            be = b.copy()
            be[1:] += b[:-1]
            be[2:] += np.where(mb[2:], b[:-2], 0.0)
            v = float((a[::-1] * be).sum())
            if not (np.isfinite(v) and v > 0.0):
                return None
            ll = np.log(v) - lacc[n] - T_ * SHIFT2
            losses[gn] = -ll / L_
    return np.float32(losses.mean())


# ----------------------------------------------------------------------------
# entry point
# ----------------------------------------------------------------------------

def kernel(preds, targets, pred_lengths, target_lengths):
    preds = np.asarray(preds, np.float32)
    targets = np.asarray(targets, np.int32)
    pred_lengths = np.asarray(pred_lengths, np.int32)
    target_lengths = np.asarray(target_lengths, np.int32)
    t2d = targets.reshape(N, L)

    fast_ok = (
        preds.shape == (T, N, C)
        and targets.shape == (N * L,)
        and np.all(pred_lengths == T)
        and np.all(target_lengths == L)
        and np.all(targets >= 1)
        and np.all(targets < C)
        and np.isfinite(preds).all()
        and np.abs(preds).max() < 8.0
    )
    if fast_ok:
        r = _run_device(preds, t2d, (T, C, L))
        if r is not None:
            return r
    return _ref_numpy(preds, t2d, pred_lengths, target_lengths)


# revision 11
# speedup vs baseline: 1.3578x; 1.0304x over previous
"""CTC loss (nn_CTCLoss) Trainium2 Bass kernel, v4.

Sharding: data-parallel over batch N across 8 cores (8 samples/core).

Per core, two decoupled device pipelines:
  * DP feed: the host pre-gathers the S=2L+1 extended-label logits per
    (chain, sample, step) into a slab qx [16, (T/2)*S] bf16 (row p<8 =
    forward chain of sample p, row p>=8 = backward chain, time- and
    state-reversed so both chains read forward).  States are stored
    de-interleaved per step: [labels (L), blanks (L+1)], which turns the
    CTC three-tap recurrence into contiguous short ops.  Two early DMAs
    (first 16-step block, then the rest) bring it in; ScalarE
    exponentiates per 16-step block (q = exp(x + SHIFT), f32 out);
    VectorE runs the UNNORMALISED DP in the probability domain:
        u  = B + shift(Lb)        [L+1 wide]   (blank update pre-mul)
        w  = u + Lb               [L wide]     (label update pre-mul)
        A' = [w | u] * q          [S wide]
    i.e. 3 short ops per time step, T/2 sequential steps, both chains
    advancing together on disjoint partitions.  No per-step softmax
    normalisation: a constant SHIFT keeps the chain inside f32 range and
    the true denominators are restored on host.
  * Denominators: the (T, NL, C) shard streams through SBUF as 16
    [128, C] f32 tiles (one 2 MB contiguous DMA each); ScalarE computes
    exp() with a fused per-row accumulate, collecting the softmax
    denominator of every (t, n) into accT [128, 16].

Final alpha/beta states plus accT go back to the host, which stitches
the chains at the midpoint in float64 and applies sum(log acc) +
T*SHIFT before the batch mean.
"""

import sys

import numpy as np

for _p in ("/root/.axon_site", "/root/.axon_site/_ro/trn_rl_repo", "/opt/trn_rl_repo"):
    if _p not in sys.path:
        sys.path.append(_p)

NCORES = 8
NL = 8                   # samples per core
TG = 16                  # time steps per [128, C] tile and per q block
BLANK = 0
PAD = 2                  # leading zero pad columns in DP tiles

# problem dims (the graded configuration)
T, N, C, L = 256, 64, 4096, 32

SHIFT2 = -1.0            # constant per-step scale: q = exp(x + SHIFT2)


def _derived(T_, C_, L_):
    S_ = 2 * L_ + 1
    NG_ = T_ // TG                     # [128, C] tile groups
    TH_ = T_ // 2                      # steps per chain
    NB_ = TH_ // TG                    # q blocks
    return S_, NG_, TH_, NB_


# ----------------------------------------------------------------------------
# host-side helpers
# ----------------------------------------------------------------------------

def _ext_labels(t2d, S_):
    ext = np.zeros((t2d.shape[0], S_), np.int64)
    ext[:, 1::2] = t2d
    return ext


def _skip_mask(ext):
    sidx = np.arange(ext.shape[1])
    return (
        (sidx[None, :] >= 2)
        & (ext != BLANK)
        & (ext != np.roll(ext, 2, axis=1))
    )


def _ref_numpy(preds, t2d, pred_lengths, target_lengths):
    """float64 port of the reference (fallback path)."""
    preds = preds.astype(np.float64)
    Tn, n = preds.shape[0], preds.shape[1]
    S_ = 2 * t2d.shape[1] + 1
    mx = preds.max(axis=2, keepdims=True)
    lp = preds - mx - np.log(np.exp(preds - mx).sum(axis=2, keepdims=True))
    ext = _ext_labels(t2d, S_)
    lpe = lp[:, np.arange(n)[:, None], ext]
    skip_ok = _skip_mask(ext)
    NEGI = -1e30
    sidx = np.arange(S_)
    valid = sidx[None, :] < (2 * target_lengths[:, None] + 1)
    alpha = np.full((n, S_), NEGI)
    alpha[:, 0] = lpe[0, :, 0]
    alpha[:, 1] = np.where(target_lengths > 0, lpe[0, :, 1], NEGI)
    alpha = np.where(valid, alpha, NEGI)

    def lse(*a):
        m = np.maximum.reduce(a)
        m = np.where(np.isfinite(m), m, 0.0)
        return m + np.log(sum(np.exp(x - m) for x in a))

    for t in range(1, Tn):
        a2 = np.concatenate([np.full((n, 1), NEGI), alpha[:, :-1]], 1)
        a3 = np.concatenate([np.full((n, 2), NEGI), alpha[:, :-2]], 1)
        a3 = np.where(skip_ok, a3, NEGI)
        new = np.where(valid, lse(alpha, a2, a3) + lpe[t], NEGI)
        active = (t < pred_lengths)[:, None]
        alpha = np.where(active, new, alpha)
    end = 2 * target_lengths
    a_last = alpha[np.arange(n), end]
    a_prev = alpha[np.arange(n), np.maximum(end - 1, 0)]
    a_prev = np.where(target_lengths > 0, a_prev, NEGI)
    nll = -lse(a_last, a_prev)
    nll = np.where(np.isfinite(nll) & (nll < 1e29), nll, 0.0)
    return np.float32(np.mean(nll / np.maximum(target_lengths, 1)))


# ----------------------------------------------------------------------------
# kernel builder
# ----------------------------------------------------------------------------

_NC_CACHE = {}


def _build(use_masks, dims):
    """use_masks=False: de-interleaved [Lb | B] DP layout (no repeated
    labels).  use_masks=True: natural state order with mask tiles."""
    T_, C_, L_ = dims
    S_, NG_, TH_, NB_ = _derived(T_, C_, L_)
    PL = PAD + L_            # start of the blank block (no-repeat layout)

    import concourse.bacc as bacc
    import concourse.tile as tile
    from concourse import mybir

    f32 = mybir.dt.float32
    bf16 = mybir.dt.bfloat16
    Act = mybir.ActivationFunctionType

    nc = bacc.Bacc("TRN2", target_bir_lowering=False, debug=False)
    # preds shard pre-tiled on host: [group, n, t16, c]; each [128, C] tile
    # load reads 128 consecutive 16KB rows (full HBM bandwidth)
    px = nc.dram_tensor("px", [NG_, NL, TG, C_], f32, kind="ExternalInput")
    # host-gathered extended-label logits in DP order (see module docstring)
    qx = nc.dram_tensor("qx", [16, TH_ * S_], bf16, kind="ExternalInput")
    if use_masks:
        maskd = nc.dram_tensor("maskd", [16, PAD + S_], f32,
                               kind="ExternalInput")
    res = nc.dram_tensor("res", [16, PAD + S_], f32, kind="ExternalOutput")
    accd = nc.dram_tensor("accd", [128, NG_], f32, kind="ExternalOutput")

    with tile.TileContext(nc) as tc:
        with (
            tc.tile_pool(name="mt", bufs=7) as mt_pool,
            tc.tile_pool(name="scr", bufs=2) as scr_pool,
            tc.tile_pool(name="single", bufs=1) as single,
        ):
            with tc.high_priority():
                # DP feed: issued first on the sync queue so its ring
                # descriptors precede the 2MB tile flood
                qxt0 = single.tile([16, TG * S_], bf16, tag="qxt0")
                nc.sync.dma_start(out=qxt0[:], in_=qx[:, 0:TG * S_])
                qxtr = single.tile([16, (TH_ - TG) * S_], bf16, tag="qxtr")
                nc.sync.dma_start(out=qxtr[:], in_=qx[:, TG * S_:])
                if use_masks:
                    msk = single.tile([16, PAD + S_], f32, tag="msk")
                    nc.sync.dma_start(out=msk[:], in_=maskd[:])

                shiftb = single.tile([16, 1], f32, tag="shiftb")
                nc.vector.memset(shiftb[:], SHIFT2)

                A = single.tile([16, PAD + S_], f32, tag="A")
                t1 = single.tile([16, PAD + S_], f32, tag="t1")
                nc.vector.memset(A[:], 0.0)
                nc.vector.memset(t1[:], 0.0)
                if use_masks:
                    am = single.tile([16, PAD + S_], f32, tag="am")
                    nc.vector.memset(am[:], 0.0)

                # q blocks: exp of the host-gathered slab
                qct = []
                for j in range(NB_):
                    qcj = single.tile([16, TG * S_], f32, tag=f"qc{j}",
                                      name=f"qc_{j}")
                    if j == 0:
                        src = qxt0[:]
                    else:
                        src = qxtr[:, (j - 1) * TG * S_:j * TG * S_]
                    nc.scalar.activation(qcj[:], src, Act.Exp,
                                         bias=shiftb[:, 0:1], scale=1.0)
                    qct.append(qcj)

            # two accumulator tiles so consecutive big exps have no WAW chain
            accTa = single.tile([128, NG_ // 2], f32, tag="accTa")
            accTb = single.tile([128, NG_ // 2], f32, tag="accTb")

            # softmax denominators: stream the shard, fused exp+row-sum
            for g in range(NG_):
                mt = mt_pool.tile([128, C_], f32, tag="mt")
                nc.sync.dma_start(out=mt[:],
                                  in_=px[g].rearrange("n t c -> (n t) c"))
                scr = scr_pool.tile([128, C_], bf16, tag="scr")
                acc_t = accTa if g % 2 == 0 else accTb
                nc.scalar.activation(scr[:], mt[:], Act.Exp,
                                     bias=0.0, scale=1.0,
                                     accum_out=acc_t[:, g // 2:g // 2 + 1])

            # DP: T/2 sequential steps, fwd chain on partitions 0-7 and
            # (time/state-reversed) bwd chain on 8-15 advancing together
            for k in range(TH_):
                qc = qct[k // TG]
                o = (k % TG) * S_
                qk = qc[:, o:o + S_]
                if k == 0:
                    if use_masks:
                        nc.vector.tensor_copy(A[:, PAD:PAD + 2], qc[:, 0:2])
                        nc.vector.tensor_mul(am[:, PAD:], A[:, PAD:],
                                             msk[:, PAD:])
                    else:
                        # A[Lb 0] = q[l0], A[B 0] = q[b0] (cols o, o+L)
                        dst = A[:, PAD:PAD + 2 * L_].rearrange(
                            "p (a b) -> p a b", b=L_)[:, :, 0]
                        src = qc[:, o:o + 2 * L_].rearrange(
                            "p (a b) -> p a b", b=L_)[:, :, 0]
                        nc.vector.tensor_copy(dst, src)
                    continue
                if use_masks:
                    # t1 = A + shift1(A); t1 += shift2(masked A); A' = t1*q
                    nc.vector.tensor_add(t1[:, PAD:], A[:, PAD:],
                                         A[:, PAD - 1:PAD + S_ - 1])
                    nc.vector.tensor_add(t1[:, PAD:], t1[:, PAD:],
                                         am[:, 0:S_])
                    nc.vector.tensor_mul(A[:, PAD:], t1[:, PAD:], qk)
                    nc.vector.tensor_mul(am[:, PAD:], A[:, PAD:],
                                         msk[:, PAD:])
                else:
                    # u = B + shift(Lb)  (blank pre-mul, L+1 wide)
                    nc.vector.tensor_add(t1[:, PL:PL + L_ + 1],
                                         A[:, PL:PL + L_ + 1],
                                         A[:, PAD - 1:PAD + L_])
                    # w = u + Lb  (label pre-mul, L wide)
                    nc.vector.tensor_add(t1[:, PAD:PAD + L_],
                                         t1[:, PL:PL + L_],
                                         A[:, PAD:PAD + L_])
                    # A' = [w | u] * q
                    nc.vector.tensor_mul(A[:, PAD:PAD + S_],
                                         t1[:, PAD:PAD + S_], qk)

            nc.scalar.dma_start(out=accd[:, 0:NG_ // 2], in_=accTa[:])
            nc.scalar.dma_start(out=accd[:, NG_ // 2:NG_], in_=accTb[:])
            nc.sync.dma_start(out=res[:], in_=A[:])
    nc.compile()
    return nc


def _get_nc(use_masks, dims):
    key = (use_masks, dims)
    if key not in _NC_CACHE:
        _NC_CACHE[key] = _build(use_masks, dims)
    return _NC_CACHE[key]


# ----------------------------------------------------------------------------
# device run for one full (T_, N=64, C_) problem
# ----------------------------------------------------------------------------

def _run_device(preds, t2d, dims):
    T_, C_, L_ = dims
    S_, NG_, TH_, NB_ = _derived(T_, C_, L_)
    import ml_dtypes

    ext = _ext_labels(t2d, S_)                    # (N, S)
    m_fwd = _skip_mask(ext)
    use_masks = bool((t2d[:, 1:] == t2d[:, :-1]).any())

    # m'[s] = m[s+2] (allowed s -> s+2); backward chain is state-reversed
    m_p = np.zeros_like(m_fwd)
    m_p[:, :-2] = m_fwd[:, 2:]
    m_bwd = m_p[:, ::-1]

    from concourse.bass_utils import run_bass_kernel_spmd

    nc = _get_nc(use_masks, dims)

    # gather column orders: natural (masks) or [labels | blanks] split
    if use_masks:
        cols_f = ext                               # (N, S)
        cols_b = ext[:, ::-1]
    else:
        cols_f = np.concatenate([ext[:, 1::2], ext[:, 0::2]], axis=1)
        extr = ext[:, ::-1]
        cols_b = np.concatenate([extr[:, 1::2], extr[:, 0::2]], axis=1)

    in_maps = []
    for c in range(NCORES):
        n0 = c * NL
        sh = preds[:, n0:n0 + NL, :]               # (T, NL, C)
        # pre-tile: (T, NL, C) -> (NG, NL, TG, C) with (n, t16) row order
        tiles = np.ascontiguousarray(
            sh.reshape(NG_, TG, NL, C_).transpose(0, 2, 1, 3))
        # host-gathered DP slab [16, TH*S]
        nidx = np.arange(NL)[:, None, None]
        kidx = np.arange(TH_)[None, :, None]
        qf = sh[kidx, nidx, cols_f[n0:n0 + NL, None, :]]         # (NL,TH,S)
        qb = sh[T_ - 1 - kidx, nidx, cols_b[n0:n0 + NL, None, :]]
        qxa = np.empty((16, TH_ * S_), np.float32)
        qxa[0:NL] = qf.reshape(NL, TH_ * S_)
        qxa[NL:16] = qb.reshape(NL, TH_ * S_)
        im = {"px": tiles, "qx": qxa.astype(ml_dtypes.bfloat16)}
        if use_masks:
            # am-premask: am[x] = A[x] * M[x+2] so that am[s-2] carries the
            # destination mask M[s]
            mam_f = np.zeros_like(m_fwd)
            mam_f[:, :-2] = m_fwd[:, 2:]
            mam_b = np.zeros_like(m_bwd)
            mam_b[:, :-2] = m_bwd[:, 2:]
            mtile = np.zeros((16, PAD + S_), np.float32)
            mtile[0:NL, PAD:] = mam_f[n0:n0 + NL]
            mtile[NL:16, PAD:] = mam_b[n0:n0 + NL]
            im["maskd"] = mtile
        in_maps.append(im)

    out = run_bass_kernel_spmd(nc, in_maps, core_ids=list(range(NCORES)))

    # host stitch (float64): combine the two chains at the midpoint and
    # restore the softmax denominators: ll = ln v - sum_t ln acc - T*SHIFT
    losses = np.zeros(NCORES * NL, np.float64)
    for c in range(NCORES):
        resv = np.asarray(out.results[c]["res"]).astype(np.float64)
        accv = np.asarray(out.results[c]["accd"]).astype(np.float64)
        if not (np.isfinite(accv).all() and (accv > 0).all()):
            return None
        lacc = np.log(accv).reshape(NL, TG, NG_).sum(axis=(1, 2))  # per n
        for n in range(NL):
            gn = c * NL + n
            ar = resv[n, PAD:]            # alpha_{TH-1}
            br = resv[NL + n, PAD:]       # beta_{TH}, reversed s order
            if use_masks:
                a, b = ar, br
            else:
                # de-interleave [labels | blanks] back to natural order
                a = np.empty(S_)
                a[1::2] = ar[0:L_]
                a[0::2] = ar[L_:S_]
                b = np.empty(S_)
                b[1::2] = br[0:L_]
                b[0::2] = br[L_:S_]
            mb = m_bwd[gn]
            be = b.copy()
            be[1:] += b[:-1]
            be[2:] += np.where(mb[2:], b[:-2], 0.0)
            v = float((a[::-1] * be).sum())
            if not (np.isfinite(v) and v > 0.0):
                return None
            ll = np.log(v) - lacc[n] - T_ * SHIFT2
            losses[gn] = -ll / L_
    return np.float32(losses.mean())


# ----------------------------------------------------------------------------
# entry point
# ----------------------------------------------------------------------------

def kernel(preds, targets, pred_lengths, target_lengths):
    preds = np.asarray(preds, np.float32)
    targets = np.asarray(targets, np.int32)
    pred_lengths = np.asarray(pred_lengths, np.int32)
    target_lengths = np.asarray(target_lengths, np.int32)
    t2d = targets.reshape(N, L)

    fast_ok = (
        preds.shape == (T, N, C)
        and targets.shape == (N * L,)
        and np.all(pred_lengths == T)
        and np.all(target_lengths == L)
        and np.all(targets >= 1)
        and np.all(targets < C)
        and np.isfinite(preds).all()
        and np.abs(preds).max() < 8.0
    )
    if fast_ok:
        r = _run_device(preds, t2d, (T, C, L))
        if r is not None:
            return r
    return _ref_numpy(preds, t2d, pred_lengths, target_lengths)


# revision 16
# speedup vs baseline: 1.3586x; 1.0006x over previous
"""CTC loss (nn_CTCLoss) Trainium2 Bass kernel, v4.

Sharding: data-parallel over batch N across 8 cores (8 samples/core).

Per core, two decoupled device pipelines:
  * DP feed: the host pre-gathers the S=2L+1 extended-label logits per
    (chain, sample, step) into a slab qx [16, (T/2)*S] bf16 (row p<8 =
    forward chain of sample p, row p>=8 = backward chain, time- and
    state-reversed so both chains read forward).  States are stored
    de-interleaved per step: [labels (L), blanks (L+1)], which turns the
    CTC three-tap recurrence into contiguous short ops.  Two early DMAs
    (first 16-step block, then the rest) bring it in; ScalarE
    exponentiates per 16-step block (q = exp(x + SHIFT), f32 out);
    VectorE runs the UNNORMALISED DP in the probability domain:
        u  = B + shift(Lb)        [L+1 wide]   (blank update pre-mul)
        w  = u + Lb               [L wide]     (label update pre-mul)
        A' = [w | u] * q          [S wide]
    i.e. 3 short ops per time step, T/2 sequential steps, both chains
    advancing together on disjoint partitions.  No per-step softmax
    normalisation: a constant SHIFT keeps the chain inside f32 range and
    the true denominators are restored on host.
  * Denominators: the (T, NL, C) shard streams through SBUF as 16
    [128, C] f32 tiles (one 2 MB contiguous DMA each); ScalarE computes
    exp() with a fused per-row accumulate, collecting the softmax
    denominator of every (t, n) into accT [128, 16].

Final alpha/beta states plus accT go back to the host, which stitches
the chains at the midpoint in float64 and applies sum(log acc) +
T*SHIFT before the batch mean.
"""

import sys

import numpy as np

for _p in ("/root/.axon_site", "/root/.axon_site/_ro/trn_rl_repo", "/opt/trn_rl_repo"):
    if _p not in sys.path:
        sys.path.append(_p)

NCORES = 8
NL = 8                   # samples per core
TG = 16                  # time steps per [128, C] tile and per q block
BLANK = 0
PAD = 2                  # leading zero pad columns in DP tiles

# problem dims (the graded configuration)
T, N, C, L = 256, 64, 4096, 32

SHIFT2 = -1.0            # constant per-step scale: q = exp(x + SHIFT2)


def _derived(T_, C_, L_):
    S_ = 2 * L_ + 1
    NG_ = T_ // TG                     # [128, C] tile groups
    TH_ = T_ // 2                      # steps per chain
    NB_ = TH_ // TG                    # q blocks
    return S_, NG_, TH_, NB_


# ----------------------------------------------------------------------------
# host-side helpers
# ----------------------------------------------------------------------------

def _ext_labels(t2d, S_):
    ext = np.zeros((t2d.shape[0], S_), np.int64)
    ext[:, 1::2] = t2d
    return ext


def _skip_mask(ext):
    sidx = np.arange(ext.shape[1])
    return (
        (sidx[None, :] >= 2)
        & (ext != BLANK)
        & (ext != np.roll(ext, 2, axis=1))
    )


def _ref_numpy(preds, t2d, pred_lengths, target_lengths):
    """float64 port of the reference (fallback path)."""
    preds = preds.astype(np.float64)
    Tn, n = preds.shape[0], preds.shape[1]
    S_ = 2 * t2d.shape[1] + 1
    mx = preds.max(axis=2, keepdims=True)
    lp = preds - mx - np.log(np.exp(preds - mx).sum(axis=2, keepdims=True))
    ext = _ext_labels(t2d, S_)
    lpe = lp[:, np.arange(n)[:, None], ext]
    skip_ok = _skip_mask(ext)
    NEGI = -1e30
    sidx = np.arange(S_)
    valid = sidx[None, :] < (2 * target_lengths[:, None] + 1)
    alpha = np.full((n, S_), NEGI)
    alpha[:, 0] = lpe[0, :, 0]
    alpha[:, 1] = np.where(target_lengths > 0, lpe[0, :, 1], NEGI)
    alpha = np.where(valid, alpha, NEGI)

    def lse(*a):
        m = np.maximum.reduce(a)
        m = np.where(np.isfinite(m), m, 0.0)
        return m + np.log(sum(np.exp(x - m) for x in a))

    for t in range(1, Tn):
        a2 = np.concatenate([np.full((n, 1), NEGI), alpha[:, :-1]], 1)
        a3 = np.concatenate([np.full((n, 2), NEGI), alpha[:, :-2]], 1)
        a3 = np.where(skip_ok, a3, NEGI)
        new = np.where(valid, lse(alpha, a2, a3) + lpe[t], NEGI)
        active = (t < pred_lengths)[:, None]
        alpha = np.where(active, new, alpha)
    end = 2 * target_lengths
    a_last = alpha[np.arange(n), end]
    a_prev = alpha[np.arange(n), np.maximum(end - 1, 0)]
    a_prev = np.where(target_lengths > 0, a_prev, NEGI)
    nll = -lse(a_last, a_prev)
    nll = np.where(np.isfinite(nll) & (nll < 1e29), nll, 0.0)
    return np.float32(np.mean(nll / np.maximum(target_lengths, 1)))


# ----------------------------------------------------------------------------
# kernel builder
# ----------------------------------------------------------------------------

_NC_CACHE = {}


def _build(use_masks, dims):
    """use_masks=False: de-interleaved [Lb | B] DP layout (no repeated
    labels).  use_masks=True: natural state order with mask tiles."""
    T_, C_, L_ = dims
    S_, NG_, TH_, NB_ = _derived(T_, C_, L_)
    PL = PAD + L_            # start of the blank block (no-repeat layout)

    import concourse.bacc as bacc
    import concourse.tile as tile
    from concourse import mybir

    f32 = mybir.dt.float32
    bf16 = mybir.dt.bfloat16
    Act = mybir.ActivationFunctionType

    nc = bacc.Bacc("TRN2", target_bir_lowering=False, debug=False)
    # preds shard pre-tiled on host: [group pair, n, t16, 2*c] (two tile
    # groups side by side per row); each [128, 2C] tile load reads 128
    # consecutive 32KB rows (full HBM bandwidth, one wait per 4MB)
    px = nc.dram_tensor("px", [NG_ // 2, NL, TG, 2 * C_], f32,
                        kind="ExternalInput")
    # host-gathered extended-label logits in DP order (see module docstring)
    qx = nc.dram_tensor("qx", [16, TH_ * S_], bf16, kind="ExternalInput")
    if use_masks:
        maskd = nc.dram_tensor("maskd", [16, PAD + S_], f32,
                               kind="ExternalInput")
    res = nc.dram_tensor("res", [16, PAD + S_], f32, kind="ExternalOutput")
    accd = nc.dram_tensor("accd", [128, NG_], f32, kind="ExternalOutput")

    with tile.TileContext(nc) as tc:
        with (
            tc.tile_pool(name="mt", bufs=3) as mt_pool,
            tc.tile_pool(name="scr", bufs=2) as scr_pool,
            tc.tile_pool(name="single", bufs=1) as single,
        ):
            with tc.high_priority():
                # DP feed: issued first on the sync queue so its ring
                # descriptors precede the 2MB tile flood
                qxt0 = single.tile([16, TG * S_], bf16, tag="qxt0")
                nc.sync.dma_start(out=qxt0[:], in_=qx[:, 0:TG * S_])
                qxtr = single.tile([16, (TH_ - TG) * S_], bf16, tag="qxtr")
                nc.sync.dma_start(out=qxtr[:], in_=qx[:, TG * S_:])
                if use_masks:
                    msk = single.tile([16, PAD + S_], f32, tag="msk")
                    nc.sync.dma_start(out=msk[:], in_=maskd[:])

                shiftb = single.tile([16, 1], f32, tag="shiftb")
                nc.vector.memset(shiftb[:], SHIFT2)
                # dummy activation: pulls ACT_TABLE_LOAD to the very start
                warm = single.tile([16, 1], f32, tag="warm")
                nc.scalar.activation(warm[:], shiftb[:], Act.Exp,
                                     bias=0.0, scale=1.0)

                A = single.tile([16, PAD + S_], f32, tag="A")
                t1 = single.tile([16, PAD + S_], f32, tag="t1")
                nc.vector.memset(A[:], 0.0)
                nc.vector.memset(t1[:], 0.0)
                if use_masks:
                    am = single.tile([16, PAD + S_], f32, tag="am")
                    nc.vector.memset(am[:], 0.0)

                # q blocks: exp of the host-gathered slab
                qct = []
                for j in range(NB_):
                    qcj = single.tile([16, TG * S_], f32, tag=f"qc{j}",
                                      name=f"qc_{j}")
                    if j == 0:
                        src = qxt0[:]
                    else:
                        src = qxtr[:, (j - 1) * TG * S_:j * TG * S_]
                    nc.scalar.activation(qcj[:], src, Act.Exp,
                                         bias=shiftb[:, 0:1], scale=1.0)
                    qct.append(qcj)

            # two accumulator tiles so consecutive big exps have no WAW chain
            accTa = single.tile([128, NG_ // 2], f32, tag="accTa")
            accTb = single.tile([128, NG_ // 2], f32, tag="accTb")

            # softmax denominators: stream the shard, fused exp+row-sum;
            # two activations per 4MB tile -> one DMA wait per pair
            for j in range(NG_ // 2):
                mt = mt_pool.tile([128, 2 * C_], f32, tag="mt")
                nc.sync.dma_start(out=mt[:],
                                  in_=px[j].rearrange("n t c -> (n t) c"))
                scr = scr_pool.tile([128, C_], bf16, tag="scr")
                nc.scalar.activation(scr[:], mt[:, 0:C_], Act.Exp,
                                     bias=0.0, scale=1.0,
                                     accum_out=accTa[:, j:j + 1])
                scr2 = scr_pool.tile([128, C_], bf16, tag="scr")
                nc.scalar.activation(scr2[:], mt[:, C_:2 * C_], Act.Exp,
                                     bias=0.0, scale=1.0,
                                     accum_out=accTb[:, j:j + 1])

            # DP: T/2 sequential steps, fwd chain on partitions 0-7 and
            # (time/state-reversed) bwd chain on 8-15 advancing together
            for k in range(TH_):
                qc = qct[k // TG]
                o = (k % TG) * S_
                qk = qc[:, o:o + S_]
                if k == 0:
                    if use_masks:
                        nc.vector.tensor_copy(A[:, PAD:PAD + 2], qc[:, 0:2])
                        nc.vector.tensor_mul(am[:, PAD:], A[:, PAD:],
                                             msk[:, PAD:])
                    else:
                        # A[Lb 0] = q[l0], A[B 0] = q[b0] (cols o, o+L)
                        dst = A[:, PAD:PAD + 2 * L_].rearrange(
                            "p (a b) -> p a b", b=L_)[:, :, 0]
                        src = qc[:, o:o + 2 * L_].rearrange(
                            "p (a b) -> p a b", b=L_)[:, :, 0]
                        nc.vector.tensor_copy(dst, src)
                    continue
                if use_masks:
                    # t1 = A + shift1(A); t1 += shift2(masked A); A' = t1*q
                    nc.vector.tensor_add(t1[:, PAD:], A[:, PAD:],
                                         A[:, PAD - 1:PAD + S_ - 1])
                    nc.vector.tensor_add(t1[:, PAD:], t1[:, PAD:],
                                         am[:, 0:S_])
                    nc.vector.tensor_mul(A[:, PAD:], t1[:, PAD:], qk)
                    nc.vector.tensor_mul(am[:, PAD:], A[:, PAD:],
                                         msk[:, PAD:])
                else:
                    # u = B + shift(Lb)  (blank pre-mul, L+1 wide)
                    nc.vector.tensor_add(t1[:, PL:PL + L_ + 1],
                                         A[:, PL:PL + L_ + 1],
                                         A[:, PAD - 1:PAD + L_])
                    # w = u + Lb  (label pre-mul, L wide)
                    nc.vector.tensor_add(t1[:, PAD:PAD + L_],
                                         t1[:, PL:PL + L_],
                                         A[:, PAD:PAD + L_])
                    # A' = [w | u] * q
                    nc.vector.tensor_mul(A[:, PAD:PAD + S_],
                                         t1[:, PAD:PAD + S_], qk)

            nc.scalar.dma_start(out=accd[:, 0:NG_ // 2], in_=accTa[:])
            nc.scalar.dma_start(out=accd[:, NG_ // 2:NG_], in_=accTb[:])
            nc.sync.dma_start(out=res[:], in_=A[:])
    nc.compile()
    return nc


def _get_nc(use_masks, dims):
    key = (use_masks, dims)
    if key not in _NC_CACHE:
        _NC_CACHE[key] = _build(use_masks, dims)
    return _NC_CACHE[key]


# ----------------------------------------------------------------------------
# device run for one full (T_, N=64, C_) problem
# ----------------------------------------------------------------------------

def _run_device(preds, t2d, dims):
    T_, C_, L_ = dims
    S_, NG_, TH_, NB_ = _derived(T_, C_, L_)
    import ml_dtypes

    ext = _ext_labels(t2d, S_)                    # (N, S)
    m_fwd = _skip_mask(ext)
    use_masks = bool((t2d[:, 1:] == t2d[:, :-1]).any())

    # m'[s] = m[s+2] (allowed s -> s+2); backward chain is state-reversed
    m_p = np.zeros_like(m_fwd)
    m_p[:, :-2] = m_fwd[:, 2:]
    m_bwd = m_p[:, ::-1]

    from concourse.bass_utils import run_bass_kernel_spmd

    nc = _get_nc(use_masks, dims)

    # gather column orders: natural (masks) or [labels | blanks] split
    if use_masks:
        cols_f = ext                               # (N, S)
        cols_b = ext[:, ::-1]
    else:
        cols_f = np.concatenate([ext[:, 1::2], ext[:, 0::2]], axis=1)
        extr = ext[:, ::-1]
        cols_b = np.concatenate([extr[:, 1::2], extr[:, 0::2]], axis=1)

    in_maps = []
    for c in range(NCORES):
        n0 = c * NL
        sh = preds[:, n0:n0 + NL, :]               # (T, NL, C)
        # pre-tile: (T, NL, C) -> (NG/2, NL, TG, 2C): group pairs side by
        # side along the free axis, (n, t16) row order
        t4 = sh.reshape(NG_, TG, NL, C_).transpose(0, 2, 1, 3)
        tiles = np.ascontiguousarray(
            np.concatenate([t4[0::2], t4[1::2]], axis=3))
        # host-gathered DP slab [16, TH*S]
        nidx = np.arange(NL)[:, None, None]
        kidx = np.arange(TH_)[None, :, None]
        qf = sh[kidx, nidx, cols_f[n0:n0 + NL, None, :]]         # (NL,TH,S)
        qb = sh[T_ - 1 - kidx, nidx, cols_b[n0:n0 + NL, None, :]]
        qxa = np.empty((16, TH_ * S_), np.float32)
        qxa[0:NL] = qf.reshape(NL, TH_ * S_)
        qxa[NL:16] = qb.reshape(NL, TH_ * S_)
        im = {"px": tiles, "qx": qxa.astype(ml_dtypes.bfloat16)}
        if use_masks:
            # am-premask: am[x] = A[x] * M[x+2] so that am[s-2] carries the
            # destination mask M[s]
            mam_f = np.zeros_like(m_fwd)
            mam_f[:, :-2] = m_fwd[:, 2:]
            mam_b = np.zeros_like(m_bwd)
            mam_b[:, :-2] = m_bwd[:, 2:]
            mtile = np.zeros((16, PAD + S_), np.float32)
            mtile[0:NL, PAD:] = mam_f[n0:n0 + NL]
            mtile[NL:16, PAD:] = mam_b[n0:n0 + NL]
            im["maskd"] = mtile
        in_maps.append(im)

    out = run_bass_kernel_spmd(nc, in_maps, core_ids=list(range(NCORES)))

    # host stitch (float64): combine the two chains at the midpoint and
    # restore the softmax denominators: ll = ln v - sum_t ln acc - T*SHIFT
    losses = np.zeros(NCORES * NL, np.float64)
    for c in range(NCORES):
        resv = np.asarray(out.results[c]["res"]).astype(np.float64)
        accv = np.asarray(out.results[c]["accd"]).astype(np.float64)
        if not (np.isfinite(accv).all() and (accv > 0).all()):
            return None
        lacc = np.log(accv).reshape(NL, TG, NG_).sum(axis=(1, 2))  # per n
        for n in range(NL):
            gn = c * NL + n
            ar = resv[n, PAD:]            # alpha_{TH-1}
            br = resv[NL + n, PAD:]       # beta_{TH}, reversed s order
            if use_masks:
                a, b = ar, br
            else:
                # de-interleave [labels | blanks] back to natural order
                a = np.empty(S_)
                a[1::2] = ar[0:L_]
                a[0::2] = ar[L_:S_]
                b = np.empty(S_)
                b[1::2] = br[0:L_]
                b[0::2] = br[L_:S_]
            mb = m_bwd[gn]
            be = b.copy()
            be[1:] += b[:-1]
            be[2:] += np.where(mb[2:], b[:-2], 0.0)
            v = float((a[::-1] * be).sum())
            if not (np.isfinite(v) and v > 0.0):
                return None
            ll = np.log(v) - lacc[n] - T_ * SHIFT2
            losses[gn] = -ll / L_
    return np.float32(losses.mean())


# ----------------------------------------------------------------------------
# entry point
# ----------------------------------------------------------------------------

def kernel(preds, targets, pred_lengths, target_lengths):
    preds = np.asarray(preds, np.float32)
    targets = np.asarray(targets, np.int32)
    pred_lengths = np.asarray(pred_lengths, np.int32)
    target_lengths = np.asarray(target_lengths, np.int32)
    t2d = targets.reshape(N, L)

    fast_ok = (
        preds.shape == (T, N, C)
        and targets.shape == (N * L,)
        and np.all(pred_lengths == T)
        and np.all(target_lengths == L)
        and np.all(targets >= 1)
        and np.all(targets < C)
        and np.isfinite(preds).all()
        and np.abs(preds).max() < 8.0
    )
    if fast_ok:
        r = _run_device(preds, t2d, (T, C, L))
        if r is not None:
            return r
    return _ref_numpy(preds, t2d, pred_lengths, target_lengths)
